# revision 1
# baseline (speedup 1.0000x reference)
"""Multi-head attention with exclusive post-processing, sharded over 8 trn2 cores.

Sharding: data-parallel over batch (2) x tensor-parallel over heads (16 -> 4/core).
Each core computes a partial transposed output [D, S] for its batch from its 4
heads; the host sums the 4 partials per batch, transposes back, and adds bo.

Device layouts are feature-major ("T" = [feature, position]) so every matmul
contraction sits on the partition axis:
  QT/KT [256, S]     <- W.T @ x.T  (bf16, head pairs stacked on partitions)
  v^T   [64, S]/head (base partition 0 so DVE ops stay partition-aligned)
  scoresT [keys, q]  <- KT_h slices.T @ QT_h
  P^T = exp(scoresT/8)   (ScalarE, scale folded into the activation)
  Y'[128, q] <- [V_h | ones].T @ P^T : rows 0..63 = unnormalized Y, rows
  64..127 = softmax denominator broadcast across partitions for free.
  Exclusive step in closed form: y_excl = (Y - (Y.v)/(sum v^2 + eps) v)/denom,
  with both reciprocals computed as exp(-ln(x)) on ScalarE (ln and exp share
  one ACT table set; DVE's iterative RECIPROCAL is ~8x slower).
  out^T[D, S] <- Wo_h.T slices @ y_excl (bf16, per-head K=64 contraction).

Phase D is split: D1 keeps PE/ACT dense (only a PSUM->SBUF copy and a Ln per
head leave the kc loop); D2 does the exclusive tail off the PE critical path,
interleaving with the next q-block's D1 and the out-projection.
"""

import os
from contextlib import ExitStack

import ml_dtypes
import numpy as np

import concourse.bass as bass
import concourse.mybir as mybir
import concourse.tile as tile
from concourse import bacc, bass_utils
from concourse.alu_op_type import AluOpType
from concourse.bass_isa import ReduceOp

F32 = mybir.dt.float32
F32R = mybir.dt.float32r
BF16 = mybir.dt.bfloat16
AF = mybir.ActivationFunctionType

B, S_FULL, D_FULL, H_FULL = 2, 2048, 1024, 16
HD = 64
N_CORES = 8
HEADS_PER_CORE = H_FULL * B // N_CORES  # 4


def build_nc(S=S_FULL, D=D_FULL, HL=HEADS_PER_CORE, use_bias=False):
    """Build the per-core Bass kernel. Returns a finalized Bacc object."""
    P = 128
    nH = HL * HD          # local fused head dim (256)
    KC = D // P           # x contraction chunks (8)
    NKc = S // P          # key chunks (16)
    QB = min(1024, S)     # q block (PSUM-sized)
    NQ = S // QB
    MT = nH // P          # feature M-tiles for QT/KT (2)
    DM = D // P           # out-proj M-tiles (8)
    NS = min(512, QB)     # matmul moving-dim chunk

    assert S % P == 0 and D % P == 0 and nH % P == 0 and QB % NS == 0

    _ensure_act_root()
    nc = bacc.Bacc(None, target_bir_lowering=False)

    xT_d = nc.dram_tensor("xT", [D, S], BF16, kind="ExternalInput")
    wq_d = nc.dram_tensor("wq", [D, nH], BF16, kind="ExternalInput")
    wk_d = nc.dram_tensor("wk", [D, nH], BF16, kind="ExternalInput")
    wv_d = nc.dram_tensor("wv", [D, nH], BF16, kind="ExternalInput")
    wo_d = nc.dram_tensor("wo", [nH, D], BF16, kind="ExternalInput")
    if use_bias:
        bq_d = nc.dram_tensor("bq", [1, nH], F32, kind="ExternalInput")
        bk_d = nc.dram_tensor("bk", [1, nH], F32, kind="ExternalInput")
        bv_d = nc.dram_tensor("bv", [1, nH], F32, kind="ExternalInput")
    outT_d = nc.dram_tensor("outT", [D, S], F32, kind="ExternalOutput")

    with tile.TileContext(nc) as tc, ExitStack() as ctx:
        consts = ctx.enter_context(tc.tile_pool(name="consts", bufs=1))
        psA = ctx.enter_context(tc.tile_pool(name="psA", bufs=2, space="PSUM"))
        psB = ctx.enter_context(tc.tile_pool(name="psB", bufs=2, space="PSUM"))
        pP = ctx.enter_context(tc.tile_pool(name="pP", bufs=4))
        ostgp = ctx.enter_context(tc.tile_pool(name="ostgp", bufs=2))
        stk = ctx.enter_context(tc.tile_pool(name="stk", bufs=2))
        bcs = ctx.enter_context(tc.tile_pool(name="bcs", bufs=2))
        bcs2 = ctx.enter_context(tc.tile_pool(name="bcs2", bufs=2))
        tps = ctx.enter_context(tc.tile_pool(name="tps", bufs=2))
        tps2 = ctx.enter_context(tc.tile_pool(name="tps2", bufs=2))
        ysbp = ctx.enter_context(tc.tile_pool(name="ysbp", bufs=6))
        lndp = ctx.enter_context(tc.tile_pool(name="lndp", bufs=5))

        # ---- ACT table preload: dummy exp+ln force the (single) table-set
        # load at kernel start, not as a 2.7us PE-stalling hiccup at the
        # start of the attention phase (which re-throttles the PE clock).
        smallc = consts.tile([P, 33], F32, tag="smallc")
        warm = smallc[0:1, 1:33]
        nc.vector.memset(warm, 1.0)
        nc.scalar.activation(out=warm, in_=warm, func=AF.Exp)
        nc.scalar.activation(out=warm, in_=warm, func=AF.Ln)

        # ---- input staging ----
        xT_sb = []
        for kc in range(KC):
            t = consts.tile([P, S], BF16, tag=f"xT{kc}")
            nc.sync.dma_start(out=t, in_=xT_d.ap()[kc * P:(kc + 1) * P, :])
            xT_sb.append(t)

        def load_w(dram):
            tiles = []
            for kc in range(KC):
                t = consts.tile([P, nH], BF16, tag=f"w{dram.name}{kc}")
                nc.sync.dma_start(out=t, in_=dram.ap()[kc * P:(kc + 1) * P, :])
                tiles.append(t)
            return tiles

        wq_sb, wk_sb, wv_sb = load_w(wq_d), load_w(wk_d), load_w(wv_d)

        wo_bf = []
        for h in range(HL):
            wbf = consts.tile([HD, D], BF16, tag=f"wobf_{h}", name=f"wobf_{h}")
            nc.sync.dma_start(out=wbf, in_=wo_d.ap()[h * HD:(h + 1) * HD, :])
            wo_bf.append(wbf)

        if use_bias:
            ones_row = consts.tile([1, max(S, P)], F32, tag="ones_row")
            nc.vector.memset(ones_row, 1.0)
            b_sb = {}
            for name, dram in (("q", bq_d), ("k", bk_d), ("v", bv_d)):
                t = consts.tile([1, nH], F32, tag=f"b{name}")
                nc.sync.dma_start(out=t, in_=dram.ap())
                b_sb[name] = t

        # eps vector for the ln(sum v^2 + eps) bias
        epsv = smallc[:, 0:1]
        nc.vector.memset(epsv, 1e-12)
        # ones64x64: all-ones [64,64] -> column-sum matmuls produce the result
        # broadcast across all 64 output partitions for free
        ones64x64 = consts.tile([HD, HD], BF16, tag="ones64x64")
        nc.vector.memset(ones64x64, 1.0)

        # ---- phase B: feature-major projections QT/KT [nH, S] (bf16, head pairs) ----
        QT = [consts.tile([P, S], BF16, tag=f"QT{t_i}", name=f"QT{t_i}") for t_i in range(MT)]
        KT = [consts.tile([P, S], BF16, tag=f"KT{t_i}", name=f"KT{t_i}") for t_i in range(MT)]

        def emit_qk(mt):
            for w_sb, dst, bias_key in ((wq_sb, QT, "q"), (wk_sb, KT, "k")):
                for qb in range(NQ):
                    ps = psA.tile([P, QB], F32, tag="ps", name="ps_qk")
                    if use_bias:
                        for ns in range(0, QB, NS):
                            nc.tensor.matmul(
                                ps[:, ns:ns + NS],
                                lhsT=b_sb[bias_key][:, mt * P:(mt + 1) * P].bitcast(F32R),
                                rhs=ones_row[:, :NS].bitcast(F32R),
                                start=True, stop=False)
                    for kc in range(KC):
                        for ns in range(0, QB, NS):
                            nc.tensor.matmul(
                                ps[:, ns:ns + NS],
                                lhsT=w_sb[kc][:, mt * P:(mt + 1) * P],
                                rhs=xT_sb[kc][:, qb * QB + ns:qb * QB + ns + NS],
                                start=(kc == 0 and not use_bias), stop=(kc == KC - 1))
                    nc.vector.tensor_copy(out=dst[mt][:, qb * QB:(qb + 1) * QB], in_=ps)

        # ---- phase B2: per-head v^T [64, S] at base partition 0 (DVE alignment) ----
        VTh = [consts.tile([HD, S], BF16, tag=f"VTh{h}", name=f"VTh{h}") for h in range(HL)]

        def emit_vth(h):
            for qb in range(NQ):
                ps = psA.tile([HD, QB], F32, tag="ps", name="ps_vth")
                if use_bias:
                    for ns in range(0, QB, NS):
                        nc.tensor.matmul(
                            ps[:, ns:ns + NS],
                            lhsT=b_sb["v"][:, h * HD:(h + 1) * HD].bitcast(F32R),
                            rhs=ones_row[:, :NS].bitcast(F32R),
                            start=True, stop=False)
                for kc in range(KC):
                    for ns in range(0, QB, NS):
                        nc.tensor.matmul(
                            ps[:, ns:ns + NS],
                            lhsT=wv_sb[kc][:, h * HD:(h + 1) * HD],
                            rhs=xT_sb[kc][:, qb * QB + ns:qb * QB + ns + NS],
                            start=(kc == 0 and not use_bias), stop=(kc == KC - 1))
                nc.vector.tensor_copy(out=VTh[h][:, qb * QB:(qb + 1) * QB], in_=ps)

        # ---- phase C: position-major V with a 64-wide ones block ----
        vprime = consts.tile([P, NKc, HL, 2 * HD], BF16, tag="vprime")

        def emit_vprime():
            nc.vector.memset(vprime[:, :, :, HD:2 * HD], 1.0)
            for qt in range(NKc):
                ps = psA.tile([P, nH], F32, tag="ps", name="ps_vp")
                if use_bias:
                    nc.tensor.matmul(
                        ps, lhsT=ones_row[:, 0:P].bitcast(F32R),
                        rhs=b_sb["v"].bitcast(F32R), start=True, stop=False)
                for kc in range(KC):
                    nc.tensor.matmul(
                        ps,
                        lhsT=xT_sb[kc][:, qt * P:(qt + 1) * P],
                        rhs=wv_sb[kc],
                        start=(kc == 0 and not use_bias), stop=(kc == KC - 1))
                nc.vector.tensor_copy(
                    out=vprime[:, qt, :, 0:HD],
                    in_=ps.rearrange("p (h d) -> p h d", h=HL))

        def head_slice(tiles, h):
            return tiles[h // 2][64 * (h % 2):64 * (h % 2) + 64, :]

        y_excl = [consts.tile([HD, S], BF16, tag=f"yx{h}", name=f"yx{h}") for h in range(HL)]

        def emit_d1(qb, h):
            q0 = qb * QB
            KTh, QTh = (head_slice(t, h) for t in (KT, QT))
            yp = psB.tile([P, QB], F32, tag="yp", name=f"yp{h}")

            def attn_v(pT, kc):
                for ns in range(0, QB, NS):
                    nc.tensor.matmul(
                        yp[:, ns:ns + NS],
                        lhsT=vprime[:, kc, h, :],
                        rhs=pT[:, ns:ns + NS],
                        start=(kc == 0), stop=(kc == NKc - 1))

            # software-pipelined by one chunk: attn@V for kc-1 is emitted after
            # scores(kc), so the exp(kc-1) wait never blocks independent score
            # matmuls behind it in the in-order PE stream
            prev = None
            for kc in range(NKc):
                sc = psA.tile([P, QB], F32, tag="ps", name=f"sc{h}")
                for ns in range(0, QB, NS):
                    nc.tensor.matmul(
                        sc[:, ns:ns + NS],
                        lhsT=KTh[:, kc * P:(kc + 1) * P],
                        rhs=QTh[:, q0 + ns:q0 + ns + NS],
                        start=True, stop=True)
                pT = pP.tile([P, QB], BF16, tag="pt", name=f"pt{h}")
                nc.scalar.activation(out=pT, in_=sc, func=AF.Exp, scale=0.125)
                if prev is not None:
                    attn_v(*prev)
                prev = (pT, kc)
            attn_v(*prev)
            ysb = ysbp.tile([HD, QB], BF16, tag="ysb", name=f"ysb{h}")
            nc.vector.tensor_copy(out=ysb, in_=yp[0:HD, :])
            lnden = lndp.tile([HD, QB], F32, tag="lnd", name=f"lnden{h}")
            nc.scalar.activation(out=lnden, in_=yp[HD:2 * HD, :], func=AF.Ln)
            return ysb, lnden

        def emit_pre(qb, h):
            """1/(sum v^2 + eps), broadcast -- independent of the attention
            output, so it runs alongside D1 and keeps D2's chain short."""
            q0 = qb * QB
            vth = VTh[h]
            vsq = stk.tile([HD, QB], BF16, tag="vsq")
            nc.vector.tensor_mul(vsq, vth[:, q0:q0 + QB], vth[:, q0:q0 + QB])
            d2B = psB.tile([HD, QB], F32, tag="yp", name="d2B")
            for ns in range(0, QB, NS):
                nc.tensor.matmul(d2B[:, ns:ns + NS], lhsT=ones64x64,
                                 rhs=vsq[:, ns:ns + NS], start=True, stop=True)
            lns = bcs2.tile([HD, QB], F32, tag="lns")
            nc.scalar.activation(out=lns, in_=d2B, func=AF.Ln, bias=epsv[0:HD, :])
            r2B = bcs2.tile([HD, QB], BF16, tag="r2b")
            nc.scalar.activation(out=r2B, in_=lns, func=AF.Exp, scale=-1.0)
            return r2B

        def heartbeat(dep):
            # tiny dependency-gated matmul (~60ns): spaces PE activity through
            # an otherwise PE-idle DVE chain so the clock-gate never sees a
            # fully-idle window and the tail keeps running at 2.4 GHz
            hb = psA.tile([HD, HD], F32, tag="ps", name="hb")
            nc.tensor.matmul(hb, lhsT=ones64x64, rhs=dep[:, 0:HD],
                             start=True, stop=True)

        def emit_d2(qb, h, ysb, lnden, r2B, hb=False):
            q0 = qb * QB
            vth = VTh[h]
            t_yv = stk.tile([HD, QB], BF16, tag="t_yv")
            nc.vector.tensor_mul(t_yv, ysb, vth[:, q0:q0 + QB])
            d1B = psB.tile([HD, QB], F32, tag="yp", name="d1B")
            for ns in range(0, QB, NS):
                nc.tensor.matmul(d1B[:, ns:ns + NS], lhsT=ones64x64,
                                 rhs=t_yv[:, ns:ns + NS], start=True, stop=True)

            betaB = bcs.tile([HD, QB], F32, tag="bet")
            nc.scalar.activation(out=betaB, in_=lnden, func=AF.Exp, scale=-1.0)

            aB = stk.tile([HD, QB], BF16, tag="ab")
            nc.vector.tensor_mul(aB, d1B[0:HD, :], r2B)
            if hb:
                heartbeat(aB)
            t2 = tps2.tile([HD, QB], BF16, tag="t2")
            nc.vector.tensor_mul(t2, vth[:, q0:q0 + QB], aB)
            u = tps.tile([HD, QB], BF16, tag="t1")
            nc.vector.tensor_sub(u, ysb, t2)
            if hb:
                heartbeat(u)
            nc.vector.tensor_mul(y_excl[h][:, q0:q0 + QB], u, betaB)

        def emit_e(qb, mt0=0, mt1=None):
            for mt in range(mt0, DM if mt1 is None else mt1):
                ps = psA.tile([P, QB], F32, tag="ps", name="ps_e")
                for h in range(HL):
                    lw = wo_bf[h][:, mt * P:(mt + 1) * P]
                    for ns in range(0, QB, NS):
                        nc.tensor.matmul(
                            ps[:, ns:ns + NS],
                            lhsT=lw,
                            rhs=y_excl[h][:, qb * QB + ns:qb * QB + ns + NS],
                            start=(h == 0), stop=(h == HL - 1))
                ostg = ostgp.tile([P, QB], F32, tag="ostg")
                nc.any.tensor_copy(out=ostg, in_=ps)
                nc.sync.dma_start(
                    out=outT_d.ap()[mt * P:(mt + 1) * P, qb * QB:(qb + 1) * QB],
                    in_=ostg)

        # ---- emission order: get the ACT-bound attention started early, then
        # feed the PE the remaining projection work to fill its dependency
        # cracks, so the PE never idles long enough to re-throttle. ----
        emit_qk(0)           # Q,K for heads 0,1
        emit_vprime()        # V' (needed by attn@V)
        saved = {}

        def d2_block(qb):
            # the 1/(sum v^2+eps) chains first: independent of the attention
            # output, they overlap the still-running D1s of the next q-block
            r2Bs = [emit_pre(qb, h) for h in range(HL)]
            for h in range(HL):
                emit_d2(qb, h, *saved[(qb, h)], r2Bs[h])

        saved[(0, 0)] = emit_d1(0, 0)
        emit_qk(1)           # Q,K heads 2,3 -- PE filler during D1 ACT stretches
        saved[(0, 1)] = emit_d1(0, 1)
        for h in range(HL):
            emit_vth(h)      # v^T per head -- more PE filler
        saved[(0, 2)] = emit_d1(0, 2)
        saved[(0, 3)] = emit_d1(0, 3)
        if NQ > 1:
            for qb in range(1, NQ):
                # spread the previous block's exclusive tails across the next
                # block's D1 stretches: each pair of chains hides behind ~20us
                # of scores instead of clustering into a PE-idle block
                saved[(qb, 0)] = emit_d1(qb, 0)
                pr0 = emit_pre(qb - 1, 0)
                pr1 = emit_pre(qb - 1, 1)
                emit_d2(qb - 1, 0, *saved[(qb - 1, 0)], pr0)
                emit_d2(qb - 1, 1, *saved[(qb - 1, 1)], pr1)
                saved[(qb, 1)] = emit_d1(qb, 1)
                pr2 = emit_pre(qb - 1, 2)
                pr3 = emit_pre(qb - 1, 3)
                emit_d2(qb - 1, 2, *saved[(qb - 1, 2)], pr2)
                emit_d2(qb - 1, 3, *saved[(qb - 1, 3)], pr3)
                saved[(qb, 2)] = emit_d1(qb, 2)
                # this q-block's first two exclusive tails run mid-kernel,
                # hidden behind the remaining D1 scores; only two chains are
                # left exposed at the very end
                r2b0 = emit_pre(qb, 0)
                r2b1 = emit_pre(qb, 1)
                emit_d2(qb, 0, *saved[(qb, 0)], r2b0)
                emit_d2(qb, 1, *saved[(qb, 1)], r2b1)
                saved[(qb, 3)] = emit_d1(qb, 3)
                emit_e(qb - 1)
            qL = NQ - 1
            r2b2 = emit_pre(qL, 2)
            r2b3 = emit_pre(qL, 3)
            emit_d2(qL, 2, *saved[(qL, 2)], r2b2, hb=True)
            emit_d2(qL, 3, *saved[(qL, 3)], r2b3, hb=True)
            emit_e(qL)
        else:
            d2_block(0)
            emit_e(0)

    nc.finalize()
    return nc


def shard_inputs(x, Wq, bq, Wk, bk, Wv, bv, Wo, bo, n_cores=N_CORES):
    """Full inputs -> per-core input maps (host-side transpose/slice/reshape)."""
    H = Wq.shape[1]
    cores_per_batch = n_cores // x.shape[0]
    hl = H // cores_per_batch
    in_maps = []
    for c in range(n_cores):
        b = c // cores_per_batch
        h0 = (c % cores_per_batch) * hl
        bf = ml_dtypes.bfloat16
        m = {
            "xT": np.ascontiguousarray(x[b].T).astype(bf),
            "wq": np.ascontiguousarray(Wq[:, h0:h0 + hl, :].reshape(Wq.shape[0], -1)).astype(bf),
            "wk": np.ascontiguousarray(Wk[:, h0:h0 + hl, :].reshape(Wk.shape[0], -1)).astype(bf),
            "wv": np.ascontiguousarray(Wv[:, h0:h0 + hl, :].reshape(Wv.shape[0], -1)).astype(bf),
            "wo": np.ascontiguousarray(Wo[h0:h0 + hl].reshape(-1, Wo.shape[2])).astype(bf),
        }
        if _use_bias(bq, bk, bv):
            m["bq"] = np.ascontiguousarray(bq[h0:h0 + hl].reshape(1, -1)).astype(np.float32)
            m["bk"] = np.ascontiguousarray(bk[h0:h0 + hl].reshape(1, -1)).astype(np.float32)
            m["bv"] = np.ascontiguousarray(bv[h0:h0 + hl].reshape(1, -1)).astype(np.float32)
        in_maps.append(m)
    return in_maps


def _use_bias(bq, bk, bv):
    return bool(np.any(bq) or np.any(bk) or np.any(bv))


_ACT_ROOT_READY = False


def _ensure_act_root():
    """Point walrus at an act-table root whose only set is
    natural_log_exp_and_others, so exp and ln share one ACT table set and the
    kernel never pays mid-stream ACT_TABLE_LOADs (which stall the PE long
    enough to re-throttle its clock)."""
    global _ACT_ROOT_READY
    if _ACT_ROOT_READY or os.environ.get("BASS_ACT_ROOT_JSON_PATH"):
        _ACT_ROOT_READY = True
        return
    import json
    import tempfile
    from neuronxcc.driver.Job import Job
    from neuronxcc.driver.jobs.support.FindActInfo import findActInfoFile

    orig = findActInfoFile(Job.getPackageDir(), "gen3")
    with open(orig) as f:
        info = json.load(f)
    keep = [e for e in info["act_func_sets"]
            if e["name"] == "natural_log_exp_and_others"]
    if not keep:  # unexpected layout -- fall back to stock tables
        _ACT_ROOT_READY = True
        return
    root = tempfile.mkdtemp(prefix="act_root_")
    src_dir = os.path.dirname(orig)
    for fn in os.listdir(src_dir):
        if fn != "act_info.json":
            os.symlink(os.path.join(src_dir, fn), os.path.join(root, fn))
    info["act_func_sets"] = keep
    with open(os.path.join(root, "act_info.json"), "w") as f:
        json.dump(info, f)
    os.environ["BASS_ACT_ROOT_JSON_PATH"] = os.path.join(root, "act_info.json")

    # Bacc preplaces InstLoadActFuncSet using concourse.hw_specs tables (it
    # reads the stock act_info directly); keep its set-id numbering in sync
    # with the custom single-set root.
    import concourse.hw_specs as hw_specs
    import concourse.bacc as bacc_mod
    _orig_tables = hw_specs.get_activation_tables

    def _single_set_tables(module_arch):
        tables = _orig_tables(module_arch)
        if "natural_log_exp_and_others" in tables:
            return {"natural_log_exp_and_others": tables["natural_log_exp_and_others"]}
        return tables

    hw_specs.get_activation_tables = _single_set_tables
    bacc_mod.get_activation_tables = _single_set_tables
    _ACT_ROOT_READY = True


_NC_CACHE = {}


def _get_nc(use_bias):
    if use_bias not in _NC_CACHE:
        _NC_CACHE[use_bias] = build_nc(use_bias=use_bias)
    return _NC_CACHE[use_bias]


def run_sharded(inputs, trace=False, trace_cores=None):
    """Run the SPMD kernel; returns (full_output, BassKernelResults)."""
    x, bo = inputs["x"], inputs["bo"]
    use_bias = _use_bias(inputs["bq"], inputs["bk"], inputs["bv"])
    _ensure_act_root()
    nc = _get_nc(use_bias)
    in_maps = shard_inputs(**inputs)
    res = bass_utils.run_bass_kernel_spmd(
        nc, in_maps, core_ids=list(range(N_CORES)),
        trace=trace, trace_cores=trace_cores)
    cores_per_batch = N_CORES // x.shape[0]
    out = np.empty_like(x)
    for b in range(x.shape[0]):
        acc = np.zeros((x.shape[2], x.shape[1]), np.float32)
        for c in range(b * cores_per_batch, (b + 1) * cores_per_batch):
            acc += res.results[c]["outT"]
        out[b] = acc.T + bo[None, :]
    return out, res


def kernel(**inputs):
    out, _ = run_sharded(inputs)
    return out



# revision 4
# speedup vs baseline: 1.2304x; 1.2304x over previous
"""Multi-head attention with exclusive post-processing, sharded over 8 trn2 cores.

Sharding: data-parallel over batch (2) x tensor-parallel over heads (16 -> 4/core).
Each core computes a partial transposed output [D, S] for its batch from its 4
heads; the host sums the 4 partials per batch, transposes back, and adds bo.

Per-core layout (feature-major "T" = [feature, position]); heads processed as
PAIRS stacked on the partition axis so DVE/ACT work runs at full 128-lane width:
  QT/KT/VT [128, S]  per pair (bf16)
  vprime   [128 pos, kc, h, v|ones]  position-major V with a 64-wide ones block
  scores: the two heads of a pair run as CONCURRENT K=64 matmuls on disjoint
    PE row groups (tile_position (0,0)/(64,0)) into one [128, 2*512] PSUM tile,
    so one [128,1024] exp covers both heads. ScalarE (the critical engine at
    ~147us of exp) sees the same element count as per-head processing; the PE
    sees half the score cost.
  attn@V: lhsT = [v | ones] per head -> rows 0:64 unnormalized Y, rows 64:128
    softmax denominator (free: PE matmul cost is independent of output rows).
  exclusive step, pair-packed: y_excl = (Y - (Y.v)/(sum v^2) v) / denom with
    both reciprocals on DVE (reciprocal_approx_fast, ~51 ULP) instead of
    exp(-ln(x)) on the busy ScalarE. Pair sums via one block-diagonal ones
    matmul (K=128).
  out-proj: per-pair K=128 contraction (wo stacked [128, D]).

Emission order: kc-streamed projections start as DMA chunks land; a queue of
small PE "filler" tasks (remaining projections, vprime chunks, out-proj
groups) is pumped one per kc inside the attention loops so the PE never idles
long enough for the HAM clock gate to re-throttle, and ScalarE stays fed.
"""

from contextlib import ExitStack

import ml_dtypes
import numpy as np

import concourse.mybir as mybir
import concourse.tile as tile
from concourse import bacc, bass_utils

F32 = mybir.dt.float32
BF16 = mybir.dt.bfloat16
AF = mybir.ActivationFunctionType

B, S_FULL, D_FULL, H_FULL = 2, 2048, 1024, 16
HD = 64
N_CORES = 8
HEADS_PER_CORE = H_FULL * B // N_CORES  # 4


def build_nc(S=S_FULL, D=D_FULL, HL=HEADS_PER_CORE):
    P = 128
    nH = HL * HD          # local fused head dim (256)
    KC = D // P           # x contraction chunks (8)
    NKc = S // P          # key chunks (16)
    QB = 512              # q block (one PSUM bank per head)
    NQ = S // QB          # 4
    NP = HL // 2          # head pairs (2)
    XH = S // 2           # x DMA column half

    nc = bacc.Bacc(None, target_bir_lowering=False)

    xT_d = nc.dram_tensor("xT", [D, S], BF16, kind="ExternalInput")
    wq_d = nc.dram_tensor("wq", [D, nH], BF16, kind="ExternalInput")
    wk_d = nc.dram_tensor("wk", [D, nH], BF16, kind="ExternalInput")
    wv_d = nc.dram_tensor("wv", [D, nH], BF16, kind="ExternalInput")
    wo_d = nc.dram_tensor("wo", [nH, D], BF16, kind="ExternalInput")
    outT_d = nc.dram_tensor("outT", [D, S], F32, kind="ExternalOutput")

    with tile.TileContext(nc) as tc, ExitStack() as ctx:
        consts = ctx.enter_context(tc.tile_pool(name="consts", bufs=1))
        psS = ctx.enter_context(tc.tile_pool(name="psS", bufs=2, space="PSUM"))
        psY = ctx.enter_context(tc.tile_pool(name="psY", bufs=2, space="PSUM"))
        psF = ctx.enter_context(tc.tile_pool(name="psF", bufs=2, space="PSUM"))
        pP = ctx.enter_context(tc.tile_pool(name="pP", bufs=5))
        ysbp = ctx.enter_context(tc.tile_pool(name="ysbp", bufs=3))
        denp = ctx.enter_context(tc.tile_pool(name="denp", bufs=3))
        betap = ctx.enter_context(tc.tile_pool(name="betap", bufs=2))
        r2p = ctx.enter_context(tc.tile_pool(name="r2p", bufs=2))
        tmpa = ctx.enter_context(tc.tile_pool(name="tmpa", bufs=2))
        tmpb = ctx.enter_context(tc.tile_pool(name="tmpb", bufs=2))
        tmpc = ctx.enter_context(tc.tile_pool(name="tmpc", bufs=2))
        ostgp = ctx.enter_context(tc.tile_pool(name="ostgp", bufs=2))

        # ---- ACT table preload: dummy exp forces the single table-set load
        # at kernel start instead of mid-attention.
        warm = consts.tile([1, 32], F32, tag="warm")
        nc.vector.memset(warm, 1.0)
        nc.scalar.activation(out=warm, in_=warm, func=AF.Exp)

        # block-diagonal ones [128,128]: one K=128 matmul sums 64-feature
        # blocks of both pair halves (result broadcast across each half)
        bdiag = consts.tile([P, P], BF16, tag="bdiag")
        nc.vector.memset(bdiag, 0.0)
        nc.vector.memset(bdiag[0:64, 0:64], 1.0)
        nc.vector.memset(bdiag[64:128, 64:128], 1.0)

        vprime = consts.tile([P, NKc, HL, 2 * HD], BF16, tag="vprime")
        nc.vector.memset(vprime[:, :, :, HD:2 * HD], 1.0)

        # ---- input staging: wk/wq first (projections can start at ~3us),
        # then x cols 0:XH (keys kc 0..7 + early vprime), wv, x cols XH:, wo.
        def load_w(dram, name):
            tiles = []
            for kc in range(KC):
                t = consts.tile([P, nH], BF16, tag=f"w{name}{kc}", name=f"w{name}{kc}")
                nc.sync.dma_start(out=t, in_=dram.ap()[kc * P:(kc + 1) * P, :])
                tiles.append(t)
            return tiles

        wk_sb = load_w(wk_d, "k")
        wq_sb = load_w(wq_d, "q")
        xT_sb = [consts.tile([P, S], BF16, tag=f"xT{kc}", name=f"xT{kc}") for kc in range(KC)]
        for kc in range(KC):
            nc.sync.dma_start(out=xT_sb[kc][:, 0:XH],
                              in_=xT_d.ap()[kc * P:(kc + 1) * P, 0:XH])
        wv_sb = load_w(wv_d, "v")
        for kc in range(KC):
            nc.sync.dma_start(out=xT_sb[kc][:, XH:S],
                              in_=xT_d.ap()[kc * P:(kc + 1) * P, XH:S])
        wo_sb = []
        for p in range(NP):
            t = consts.tile([P, D], BF16, tag=f"wo{p}", name=f"wo{p}")
            nc.sync.dma_start(out=t, in_=wo_d.ap()[p * P:(p + 1) * P, :])
            wo_sb.append(t)

        # ---- persistent feature-major tensors ----
        QT = [consts.tile([P, S], BF16, tag=f"QT{p}", name=f"QT{p}") for p in range(NP)]
        KT = [consts.tile([P, S], BF16, tag=f"KT{p}", name=f"KT{p}") for p in range(NP)]
        VT = [consts.tile([P, S], BF16, tag=f"VT{p}", name=f"VT{p}") for p in range(NP)]
        y_excl = [consts.tile([P, S], BF16, tag=f"yx{p}", name=f"yx{p}") for p in range(NP)]

        # ---- small task emitters (run as PE fillers) ----
        def proj_task(w_sb, dst, p, cb):
            """dst[p][:, cb*QB:(cb+1)*QB] = W_pair.T @ x chunk (K=128 x 8)."""
            ps = psF.tile([P, QB], F32, tag="f", name=f"pj{p}{cb}")
            for kc in range(KC):
                nc.tensor.matmul(
                    ps,
                    lhsT=w_sb[kc][:, p * P:(p + 1) * P],
                    rhs=xT_sb[kc][:, cb * QB:(cb + 1) * QB],
                    start=(kc == 0), stop=(kc == KC - 1))
            nc.vector.tensor_copy(out=dst[p][:, cb * QB:(cb + 1) * QB], in_=ps)

        def vprime_task(qt):
            ps = psF.tile([P, nH], F32, tag="f", name=f"vp{qt}")
            for kc in range(KC):
                nc.tensor.matmul(
                    ps,
                    lhsT=xT_sb[kc][:, qt * P:(qt + 1) * P],
                    rhs=wv_sb[kc],
                    start=(kc == 0), stop=(kc == KC - 1))
            nc.vector.tensor_copy(
                out=vprime[:, qt, :, 0:HD],
                in_=ps.rearrange("p (h d) -> p h d", h=HL))

        def e_task(qb, mt):
            """out-proj m-tile: K=128 per pair, accumulated over both pairs."""
            q0 = qb * QB
            ps = psF.tile([P, QB], F32, tag="f", name=f"e{qb}{mt}")
            for p in range(NP):
                nc.tensor.matmul(
                    ps,
                    lhsT=wo_sb[p][:, mt * P:(mt + 1) * P],
                    rhs=y_excl[p][:, q0:q0 + QB],
                    start=(p == 0), stop=(p == NP - 1))
            ostg = ostgp.tile([P, QB], F32, tag="ostg", name="ostg")
            nc.any.tensor_copy(out=ostg, in_=ps)
            nc.sync.dma_start(
                out=outT_d.ap()[mt * P:(mt + 1) * P, q0:q0 + QB], in_=ostg)

        # filler task queue: tasks are emitted either opportunistically (pump,
        # one per kc inside attention loops) or on demand (require, when a
        # consumer is about to be emitted) -- emission order defines the
        # dependencies Tile sees, so a consumer must never precede its task.
        fillers = []
        tasks = {}

        def pump(n=1):
            for _ in range(n):
                while fillers:
                    key = fillers.pop(0)
                    fn = tasks.pop(key, None)
                    if fn is not None:
                        fn()
                        break

        def require(key):
            fn = tasks.pop(key, None)
            if fn is not None:
                fn()

        def add_task(key, fn):
            tasks[key] = fn
            fillers.append(key)

        # ---- D1: scores + exp + attn@V for one (q-block, pair) ----
        def emit_d1(qb, p):
            q0 = qb * QB
            KTp, QTp = KT[p], QT[p]
            yp = [psY.tile([HD * 2, QB], F32, tag="y", name=f"yp{p}{h}")
                  for h in range(2)]

            def attn_v(pT, kc):
                for h in range(2):
                    nc.tensor.matmul(
                        yp[h],
                        lhsT=vprime[:, kc, 2 * p + h, :],
                        rhs=pT[:, h * QB:(h + 1) * QB],
                        start=(kc == 0), stop=(kc == NKc - 1))

            require(("Q", p, qb))
            prev = None
            for kc in range(NKc):
                require(("K", p, kc // (NKc // NQ)))
                require(("vp", kc))
                sc = psS.tile([P, 2 * QB], F32, tag="s", name=f"sc{p}")
                # the two heads' K=64 score matmuls run concurrently on
                # disjoint PE row groups (lhsT/rhs at base 0 vs 64)
                for h in range(2):
                    nc.tensor.matmul(
                        sc[:, h * QB:(h + 1) * QB],
                        lhsT=KTp[h * HD:(h + 1) * HD, kc * P:(kc + 1) * P],
                        rhs=QTp[h * HD:(h + 1) * HD, q0:q0 + QB],
                        start=True, stop=True)
                pT = pP.tile([P, 2 * QB], BF16, tag="pt", name=f"pt{p}")
                nc.scalar.activation(out=pT, in_=sc, func=AF.Exp, scale=0.125)
                if prev is not None:
                    attn_v(*prev)
                prev = (pT, kc)
                pump(1)
            attn_v(*prev)

            ysb = ysbp.tile([P, QB], BF16, tag="ysb", name="ysb")
            den = denp.tile([P, QB], F32, tag="den", name="den")
            nc.vector.tensor_copy(out=ysb[0:64, :], in_=yp[0][0:64, :])
            nc.vector.tensor_copy(out=ysb[64:128, :], in_=yp[1][0:64, :])
            nc.vector.tensor_copy(out=den[0:64, :], in_=yp[0][64:128, :])
            nc.vector.tensor_copy(out=den[64:128, :], in_=yp[1][64:128, :])
            return ysb, den

        # ---- D2: pair-packed exclusive step ----
        def emit_d2(qb, p, ysb, den):
            require(("V", p, qb))
            q0 = qb * QB
            vtp = VT[p][:, q0:q0 + QB]
            beta = betap.tile([P, QB], F32, tag="beta", name="beta")
            nc.vector.reciprocal_approx_fast(out=beta, in_=den)
            vsq = tmpa.tile([P, QB], BF16, tag="vsq", name="vsq")
            nc.vector.tensor_mul(vsq, vtp, vtp)
            d2B = psF.tile([P, QB], F32, tag="f", name="d2B")
            nc.tensor.matmul(d2B, lhsT=bdiag, rhs=vsq, start=True, stop=True)
            r2 = r2p.tile([P, QB], F32, tag="r2", name="r2")
            nc.vector.reciprocal_approx_fast(out=r2, in_=d2B)
            t_yv = tmpa.tile([P, QB], BF16, tag="tyv", name="tyv")
            nc.vector.tensor_mul(t_yv, ysb, vtp)
            d1B = psF.tile([P, QB], F32, tag="f", name="d1B")
            nc.tensor.matmul(d1B, lhsT=bdiag, rhs=t_yv, start=True, stop=True)
            aB = tmpb.tile([P, QB], BF16, tag="ab", name="ab")
            nc.vector.tensor_mul(aB, d1B, r2)
            t2 = tmpc.tile([P, QB], BF16, tag="t2", name="t2")
            nc.vector.tensor_mul(t2, vtp, aB)
            u = tmpb.tile([P, QB], BF16, tag="u", name="u")
            nc.vector.tensor_sub(u, ysb, t2)
            nc.vector.tensor_mul(y_excl[p][:, q0:q0 + QB], u, beta)

        # ---- emission ----
        # upfront: keys for pair0 (kc 0..7 need x half0), q-block 0, vprime
        proj_task(wk_sb, KT, 0, 0)
        proj_task(wq_sb, QT, 0, 0)
        vprime_task(0)
        vprime_task(1)
        # filler queue in rough consumption order (requires enforce deps)
        for qt in range(2, 8):
            add_task(("vp", qt), lambda qt=qt: vprime_task(qt))
        add_task(("K", 0, 1), lambda: proj_task(wk_sb, KT, 0, 1))
        for qt in range(8, NKc):
            add_task(("vp", qt), lambda qt=qt: vprime_task(qt))
        add_task(("K", 0, 2), lambda: proj_task(wk_sb, KT, 0, 2))
        add_task(("K", 0, 3), lambda: proj_task(wk_sb, KT, 0, 3))
        add_task(("V", 0, 0), lambda: proj_task(wv_sb, VT, 0, 0))
        for cb in range(1, NQ):
            add_task(("Q", 0, cb), lambda cb=cb: proj_task(wq_sb, QT, 0, cb))
            add_task(("V", 0, cb), lambda cb=cb: proj_task(wv_sb, VT, 0, cb))
        for cb in range(NQ):
            add_task(("K", 1, cb), lambda cb=cb: proj_task(wk_sb, KT, 1, cb))
        for cb in range(NQ):
            add_task(("Q", 1, cb), lambda cb=cb: proj_task(wq_sb, QT, 1, cb))
            add_task(("V", 1, cb), lambda cb=cb: proj_task(wv_sb, VT, 1, cb))

        # pair 0 over all q-blocks, then pair 1; exclusive chain right after
        # each block; out-proj groups appended as fillers once both pairs of
        # a q-block are done (consumed during later attention loops).
        for p in range(NP):
            for qb in range(NQ):
                ysb, den = emit_d1(qb, p)
                emit_d2(qb, p, ysb, den)
                if p == NP - 1:
                    if qb < NQ - 1:
                        for mt in range(D // P):
                            add_task(("e", qb, mt),
                                     lambda qb=qb, mt=mt: e_task(qb, mt))
        # drain remaining fillers, then the last q-block's out-proj
        while fillers:
            pump(1)
        for mt in range(D // P):
            e_task(NQ - 1, mt)

    nc.finalize()
    return nc


def shard_inputs(x, Wq, bq, Wk, bk, Wv, bv, Wo, bo, n_cores=N_CORES):
    """Full inputs -> per-core input maps (host-side transpose/slice/reshape)."""
    assert not (np.any(bq) or np.any(bk) or np.any(bv)), "nonzero qkv bias unsupported"
    H = Wq.shape[1]
    cores_per_batch = n_cores // x.shape[0]
    hl = H // cores_per_batch
    bf = ml_dtypes.bfloat16
    in_maps = []
    for c in range(n_cores):
        b = c // cores_per_batch
        h0 = (c % cores_per_batch) * hl
        in_maps.append({
            "xT": np.ascontiguousarray(x[b].T).astype(bf),
            "wq": np.ascontiguousarray(Wq[:, h0:h0 + hl, :].reshape(Wq.shape[0], -1)).astype(bf),
            "wk": np.ascontiguousarray(Wk[:, h0:h0 + hl, :].reshape(Wk.shape[0], -1)).astype(bf),
            "wv": np.ascontiguousarray(Wv[:, h0:h0 + hl, :].reshape(Wv.shape[0], -1)).astype(bf),
            "wo": np.ascontiguousarray(Wo[h0:h0 + hl].reshape(-1, Wo.shape[2])).astype(bf),
        })
    return in_maps


_NC_CACHE = {}


def _get_nc():
    if "nc" not in _NC_CACHE:
        _NC_CACHE["nc"] = build_nc()
    return _NC_CACHE["nc"]


def run_sharded(inputs, trace=False, trace_cores=None):
    """Run the SPMD kernel; returns (full_output, BassKernelResults)."""
    x, bo = inputs["x"], inputs["bo"]
    nc = _get_nc()
    in_maps = shard_inputs(**inputs)
    res = bass_utils.run_bass_kernel_spmd(
        nc, in_maps, core_ids=list(range(N_CORES)),
        trace=trace, trace_cores=trace_cores)
    cores_per_batch = N_CORES // x.shape[0]
    out = np.empty_like(x)
    for b in range(x.shape[0]):
        acc = np.zeros((x.shape[2], x.shape[1]), np.float32)
        for c in range(b * cores_per_batch, (b + 1) * cores_per_batch):
            acc += res.results[c]["outT"]
        out[b] = acc.T + bo[None, :]
    return out, res


def kernel(**inputs):
    out, _ = run_sharded(inputs)
    return out


# revision 7
# speedup vs baseline: 1.2441x; 1.0111x over previous
"""Multi-head attention with exclusive post-processing, sharded over 8 trn2 cores.

Sharding: data-parallel over batch (2) x tensor-parallel over heads (16 -> 4/core).
Each core computes a partial transposed output [D, S] for its batch from its 4
heads; the host sums the 4 partials per batch, transposes back, and adds bo.

Per-core layout (feature-major "T" = [feature, position]); heads processed as
PAIRS stacked on the partition axis so DVE/ACT work runs at full 128-lane width:
  QT/KT/VT [128, S]  per pair (bf16)
  vprime   [128 pos, kc, h, v|ones]  position-major V with a 64-wide ones block
  scores: the two heads of a pair run as CONCURRENT K=64 matmuls on disjoint
    PE row groups (tile_position (0,0)/(64,0)) into one [128, 2*512] PSUM tile,
    so one [128,1024] exp covers both heads. ScalarE (the critical engine at
    ~147us of exp) sees the same element count as per-head processing; the PE
    sees half the score cost.
  attn@V: lhsT = [v | ones] per head -> rows 0:64 unnormalized Y, rows 64:128
    softmax denominator (free: PE matmul cost is independent of output rows).
  exclusive step, pair-packed: y_excl = (Y - (Y.v)/(sum v^2) v) / denom with
    both reciprocals on DVE (reciprocal_approx_fast, ~51 ULP) instead of
    exp(-ln(x)) on the busy ScalarE. Pair sums via one block-diagonal ones
    matmul (K=128).
  out-proj: per-pair K=128 contraction (wo stacked [128, D]).

Emission order: kc-streamed projections start as DMA chunks land; a queue of
small PE "filler" tasks (remaining projections, vprime chunks, out-proj
groups) is pumped one per kc inside the attention loops so the PE never idles
long enough for the HAM clock gate to re-throttle, and ScalarE stays fed.
"""

from contextlib import ExitStack

import ml_dtypes
import numpy as np

import concourse.mybir as mybir
import concourse.tile as tile
from concourse import bacc, bass_utils

F32 = mybir.dt.float32
BF16 = mybir.dt.bfloat16
AF = mybir.ActivationFunctionType

B, S_FULL, D_FULL, H_FULL = 2, 2048, 1024, 16
HD = 64
N_CORES = 8
HEADS_PER_CORE = H_FULL * B // N_CORES  # 4


def build_nc(S=S_FULL, D=D_FULL, HL=HEADS_PER_CORE):
    P = 128
    nH = HL * HD          # local fused head dim (256)
    KC = D // P           # x contraction chunks (8)
    NKc = S // P          # key chunks (16)
    QB = 512              # q block (one PSUM bank per head)
    NQ = S // QB          # 4
    NP = HL // 2          # head pairs (2)
    XH = S // 2           # x DMA column half

    nc = bacc.Bacc(None, target_bir_lowering=False)

    xT_d = nc.dram_tensor("xT", [D, S], BF16, kind="ExternalInput")
    wq_d = nc.dram_tensor("wq", [D, nH], BF16, kind="ExternalInput")
    wk_d = nc.dram_tensor("wk", [D, nH], BF16, kind="ExternalInput")
    wv_d = nc.dram_tensor("wv", [D, nH], BF16, kind="ExternalInput")
    wo_d = nc.dram_tensor("wo", [nH, D], BF16, kind="ExternalInput")
    id_d = nc.dram_tensor("ident", [P, P], BF16, kind="ExternalInput")
    outT_d = nc.dram_tensor("outT", [D, S], F32, kind="ExternalOutput")

    with tile.TileContext(nc) as tc, ExitStack() as ctx:
        consts = ctx.enter_context(tc.tile_pool(name="consts", bufs=1))
        psS = ctx.enter_context(tc.tile_pool(name="psS", bufs=2, space="PSUM"))
        psY = ctx.enter_context(tc.tile_pool(name="psY", bufs=2, space="PSUM"))
        psF = ctx.enter_context(tc.tile_pool(name="psF", bufs=2, space="PSUM"))
        pP = ctx.enter_context(tc.tile_pool(name="pP", bufs=5))
        ysbp = ctx.enter_context(tc.tile_pool(name="ysbp", bufs=3))
        denp = ctx.enter_context(tc.tile_pool(name="denp", bufs=3))
        betap = ctx.enter_context(tc.tile_pool(name="betap", bufs=2))
        r2p = ctx.enter_context(tc.tile_pool(name="r2p", bufs=2))
        tmpa = ctx.enter_context(tc.tile_pool(name="tmpa", bufs=2))
        tmpb = ctx.enter_context(tc.tile_pool(name="tmpb", bufs=2))
        tmpc = ctx.enter_context(tc.tile_pool(name="tmpc", bufs=2))
        ostgp = ctx.enter_context(tc.tile_pool(name="ostgp", bufs=2))

        # ---- ACT table preload: dummy exp forces the single table-set load
        # at kernel start instead of mid-attention.
        warm = consts.tile([1, 32], F32, tag="warm")
        nc.vector.memset(warm, 1.0)
        nc.scalar.activation(out=warm, in_=warm, func=AF.Exp)

        # block-diagonal ones [128,128]: one K=128 matmul sums 64-feature
        # blocks of both pair halves (result broadcast across each half)
        bdiag = consts.tile([P, P], BF16, tag="bdiag")
        nc.vector.memset(bdiag, 0.0)
        nc.vector.memset(bdiag[0:64, 0:64], 1.0)
        nc.vector.memset(bdiag[64:128, 64:128], 1.0)

        vprime = consts.tile([P, NKc, HL, 2 * HD], BF16, tag="vprime")
        nc.vector.memset(vprime[:, :, :, HD:2 * HD], 1.0)

        # ---- input staging, split across the two HW DGE queues (sync and
        # scalar) so the 6.3MB input stream runs at ~2x one queue's ~200GB/s.
        # All scalar-queue DMAs are issued at the head, before the exp stream.
        def load_w(dram, name, eng):
            tiles = []
            for kc in range(KC):
                t = consts.tile([P, nH], BF16, tag=f"w{name}{kc}", name=f"w{name}{kc}")
                eng.dma_start(out=t, in_=dram.ap()[kc * P:(kc + 1) * P, :])
                tiles.append(t)
            return tiles

        xT_sb = [consts.tile([P, S], BF16, tag=f"xT{kc}", name=f"xT{kc}") for kc in range(KC)]

        def load_x(cb, eng):
            c0 = cb * QB
            for kc in range(KC):
                eng.dma_start(out=xT_sb[kc][:, c0:c0 + QB],
                              in_=xT_d.ap()[kc * P:(kc + 1) * P, c0:c0 + QB])

        ident = consts.tile([P, P], BF16, tag="ident")
        nc.sync.dma_start(out=ident, in_=id_d.ap())
        wk_sb = load_w(wk_d, "k", nc.sync)          # sync:   wk, x cb0, x cb1
        wq_sb = load_w(wq_d, "q", nc.scalar)        # scalar: wq, wv, x cb2, cb3, wo
        wv_sb = load_w(wv_d, "v", nc.scalar)
        load_x(0, nc.sync)
        load_x(2, nc.scalar)
        load_x(1, nc.sync)
        load_x(3, nc.scalar)
        wo_sb = []
        for p in range(NP):
            t = consts.tile([P, D], BF16, tag=f"wo{p}", name=f"wo{p}")
            nc.scalar.dma_start(out=t, in_=wo_d.ap()[p * P:(p + 1) * P, :])
            wo_sb.append(t)

        # ---- persistent feature-major tensors ----
        QT = [consts.tile([P, S], BF16, tag=f"QT{p}", name=f"QT{p}") for p in range(NP)]
        KT = [consts.tile([P, S], BF16, tag=f"KT{p}", name=f"KT{p}") for p in range(NP)]
        VT = [consts.tile([P, S], BF16, tag=f"VT{p}", name=f"VT{p}") for p in range(NP)]
        y_excl = [consts.tile([P, S], BF16, tag=f"yx{p}", name=f"yx{p}") for p in range(NP)]

        # ---- small task emitters (run as PE fillers, a ~0.5us step at a
        # time so ScalarE never waits behind a long PE filler burst) ----
        def proj_task(w_sb, dst, p, cb):
            """dst[p][:, cb*QB:(cb+1)*QB] = W_pair.T @ x chunk (K=128 x 8)."""
            ps = psF.tile([P, QB], F32, tag="f", name=f"pj{p}{cb}")
            for kc in range(KC):
                nc.tensor.matmul(
                    ps,
                    lhsT=w_sb[kc][:, p * P:(p + 1) * P],
                    rhs=xT_sb[kc][:, cb * QB:(cb + 1) * QB],
                    start=(kc == 0), stop=(kc == KC - 1))
                if kc % 2 == 1 and kc < KC - 1:
                    yield
            nc.vector.tensor_copy(out=dst[p][:, cb * QB:(cb + 1) * QB], in_=ps)

        def vprime_task(p, kc):
            """position-major V chunk via PE transpose of the feature-major
            VT block -- ~0.3us instead of an 8-matmul projection."""
            require(("V", p, kc // (NKc // NQ)))
            ps = psF.tile([P, P], BF16, tag="f", name=f"tr{p}{kc}")
            nc.tensor.transpose(ps, VT[p][:, kc * P:(kc + 1) * P], ident)
            nc.vector.tensor_copy(out=vprime[:, kc, 2 * p, 0:HD], in_=ps[:, 0:HD])
            nc.vector.tensor_copy(out=vprime[:, kc, 2 * p + 1, 0:HD],
                                  in_=ps[:, HD:2 * HD])
            return
            yield

        def e_task(qb, mt):
            """out-proj m-tile: K=128 per pair, accumulated over both pairs."""
            q0 = qb * QB
            if False:
                yield
            ps = psF.tile([P, QB], F32, tag="f", name=f"e{qb}{mt}")
            for p in range(NP):
                nc.tensor.matmul(
                    ps,
                    lhsT=wo_sb[p][:, mt * P:(mt + 1) * P],
                    rhs=y_excl[p][:, q0:q0 + QB],
                    start=(p == 0), stop=(p == NP - 1))
            ostg = ostgp.tile([P, QB], F32, tag="ostg", name="ostg")
            nc.any.tensor_copy(out=ostg, in_=ps)
            nc.sync.dma_start(
                out=outT_d.ap()[mt * P:(mt + 1) * P, q0:q0 + QB], in_=ostg)

        # filler task queue: generator tasks are advanced one ~0.5us step at
        # a time (pump, once per kc inside attention loops) or run to
        # completion on demand (require, when a consumer is about to be
        # emitted) -- emission order defines the dependencies Tile sees, so a
        # consumer must never precede its producer task.
        fillers = []
        tasks = {}

        def pump(n=1):
            for _ in range(n):
                while fillers:
                    key = fillers[0]
                    g = tasks.get(key)
                    if g is None:
                        fillers.pop(0)
                        continue
                    try:
                        next(g)
                    except StopIteration:
                        tasks.pop(key, None)
                        fillers.pop(0)
                    break

        def require(key):
            g = tasks.pop(key, None)
            if g is not None:
                for _ in g:
                    pass

        def add_task(key, gen_fn):
            tasks[key] = gen_fn()
            fillers.append(key)

        def run_task(gen_fn):
            for _ in gen_fn():
                pass

        # ---- D1: scores + exp + attn@V for one (q-block, pair) ----
        def emit_d1(qb, p):
            q0 = qb * QB
            KTp, QTp = KT[p], QT[p]
            yp = [psY.tile([HD * 2, QB], F32, tag="y", name=f"yp{p}{h}")
                  for h in range(2)]

            def attn_v(pT, kc):
                for h in range(2):
                    nc.tensor.matmul(
                        yp[h],
                        lhsT=vprime[:, kc, 2 * p + h, :],
                        rhs=pT[:, h * QB:(h + 1) * QB],
                        start=(kc == 0), stop=(kc == NKc - 1))

            require(("Q", p, qb))
            prev = None
            for kc in range(NKc):
                require(("K", p, kc // (NKc // NQ)))
                require(("vp", p, kc))
                sc = psS.tile([P, 2 * QB], F32, tag="s", name=f"sc{p}")
                # the two heads' K=64 score matmuls run concurrently on
                # disjoint PE row groups (lhsT/rhs at base 0 vs 64)
                for h in range(2):
                    nc.tensor.matmul(
                        sc[:, h * QB:(h + 1) * QB],
                        lhsT=KTp[h * HD:(h + 1) * HD, kc * P:(kc + 1) * P],
                        rhs=QTp[h * HD:(h + 1) * HD, q0:q0 + QB],
                        start=True, stop=True)
                pT = pP.tile([P, 2 * QB], BF16, tag="pt", name=f"pt{p}")
                nc.scalar.activation(out=pT, in_=sc, func=AF.Exp, scale=0.125)
                if prev is not None:
                    attn_v(*prev)
                prev = (pT, kc)
                pump(1)
            attn_v(*prev)

            ysb = ysbp.tile([P, QB], BF16, tag="ysb", name="ysb")
            den = denp.tile([P, QB], F32, tag="den", name="den")
            nc.vector.tensor_copy(out=ysb[0:64, :], in_=yp[0][0:64, :])
            nc.vector.tensor_copy(out=ysb[64:128, :], in_=yp[1][0:64, :])
            nc.vector.tensor_copy(out=den[0:64, :], in_=yp[0][64:128, :])
            nc.vector.tensor_copy(out=den[64:128, :], in_=yp[1][64:128, :])
            return ysb, den

        # ---- D2: pair-packed exclusive step ----
        def emit_d2(qb, p, ysb, den):
            require(("V", p, qb))
            q0 = qb * QB
            vtp = VT[p][:, q0:q0 + QB]
            beta = betap.tile([P, QB], F32, tag="beta", name="beta")
            nc.vector.reciprocal_approx_fast(out=beta, in_=den)
            vsq = tmpa.tile([P, QB], BF16, tag="vsq", name="vsq")
            nc.vector.tensor_mul(vsq, vtp, vtp)
            d2B = psF.tile([P, QB], F32, tag="f", name="d2B")
            nc.tensor.matmul(d2B, lhsT=bdiag, rhs=vsq, start=True, stop=True)
            r2 = r2p.tile([P, QB], F32, tag="r2", name="r2")
            nc.vector.reciprocal_approx_fast(out=r2, in_=d2B)
            t_yv = tmpa.tile([P, QB], BF16, tag="tyv", name="tyv")
            nc.vector.tensor_mul(t_yv, ysb, vtp)
            d1B = psF.tile([P, QB], F32, tag="f", name="d1B")
            nc.tensor.matmul(d1B, lhsT=bdiag, rhs=t_yv, start=True, stop=True)
            aB = tmpb.tile([P, QB], BF16, tag="ab", name="ab")
            nc.vector.tensor_mul(aB, d1B, r2)
            t2 = tmpc.tile([P, QB], BF16, tag="t2", name="t2")
            nc.vector.tensor_mul(t2, vtp, aB)
            u = tmpb.tile([P, QB], BF16, tag="u", name="u")
            nc.vector.tensor_sub(u, ysb, t2)
            nc.vector.tensor_mul(y_excl[p][:, q0:q0 + QB], u, beta)

        # ---- emission ----
        # upfront: keys/queries/values for pair0 q-block 0 and the first
        # vprime chunks; everything else queues as fillers (requires enforce
        # dependency order regardless of pump progress).
        run_task(lambda: proj_task(wk_sb, KT, 0, 0))
        run_task(lambda: proj_task(wq_sb, QT, 0, 0))
        run_task(lambda: proj_task(wv_sb, VT, 0, 0))
        run_task(lambda: vprime_task(0, 0))
        run_task(lambda: vprime_task(0, 1))
        for kc in range(2, 4):
            add_task(("vp", 0, kc), lambda kc=kc: vprime_task(0, kc))
        add_task(("K", 0, 1), lambda: proj_task(wk_sb, KT, 0, 1))
        for kc in range(4, 8):
            add_task(("vp", 0, kc), lambda kc=kc: vprime_task(0, kc))
        add_task(("K", 0, 2), lambda: proj_task(wk_sb, KT, 0, 2))
        add_task(("V", 0, 2), lambda: proj_task(wv_sb, VT, 0, 2))
        for kc in range(8, 12):
            add_task(("vp", 0, kc), lambda kc=kc: vprime_task(0, kc))
        add_task(("K", 0, 3), lambda: proj_task(wk_sb, KT, 0, 3))
        add_task(("V", 0, 3), lambda: proj_task(wv_sb, VT, 0, 3))
        for kc in range(12, NKc):
            add_task(("vp", 0, kc), lambda kc=kc: vprime_task(0, kc))
        add_task(("V", 0, 1), lambda: proj_task(wv_sb, VT, 0, 1))
        for cb in range(1, NQ):
            add_task(("Q", 0, cb), lambda cb=cb: proj_task(wq_sb, QT, 0, cb))
        for cb in range(NQ):
            add_task(("K", 1, cb), lambda cb=cb: proj_task(wk_sb, KT, 1, cb))
            add_task(("V", 1, cb), lambda cb=cb: proj_task(wv_sb, VT, 1, cb))
        for kc in range(NKc):
            add_task(("vp", 1, kc), lambda kc=kc: vprime_task(1, kc))
        for cb in range(NQ):
            add_task(("Q", 1, cb), lambda cb=cb: proj_task(wq_sb, QT, 1, cb))

        # pair 0 over all q-blocks, then pair 1; exclusive chain right after
        # each block; out-proj groups appended as fillers once both pairs of
        # a q-block are done (consumed during later attention loops).
        for p in range(NP):
            for qb in range(NQ):
                ysb, den = emit_d1(qb, p)
                emit_d2(qb, p, ysb, den)
                if p == NP - 1:
                    if qb < NQ - 1:
                        for mt in range(D // P):
                            add_task(("e", qb, mt),
                                     lambda qb=qb, mt=mt: e_task(qb, mt))
        # drain remaining fillers, then the last q-block's out-proj
        while fillers:
            pump(1)
        for mt in range(D // P):
            run_task(lambda mt=mt: e_task(NQ - 1, mt))

    nc.finalize()
    return nc


def shard_inputs(x, Wq, bq, Wk, bk, Wv, bv, Wo, bo, n_cores=N_CORES):
    """Full inputs -> per-core input maps (host-side transpose/slice/reshape)."""
    assert not (np.any(bq) or np.any(bk) or np.any(bv)), "nonzero qkv bias unsupported"
    H = Wq.shape[1]
    cores_per_batch = n_cores // x.shape[0]
    hl = H // cores_per_batch
    bf = ml_dtypes.bfloat16
    in_maps = []
    for c in range(n_cores):
        b = c // cores_per_batch
        h0 = (c % cores_per_batch) * hl
        in_maps.append({
            "xT": np.ascontiguousarray(x[b].T).astype(bf),
            "wq": np.ascontiguousarray(Wq[:, h0:h0 + hl, :].reshape(Wq.shape[0], -1)).astype(bf),
            "wk": np.ascontiguousarray(Wk[:, h0:h0 + hl, :].reshape(Wk.shape[0], -1)).astype(bf),
            "wv": np.ascontiguousarray(Wv[:, h0:h0 + hl, :].reshape(Wv.shape[0], -1)).astype(bf),
            "wo": np.ascontiguousarray(Wo[h0:h0 + hl].reshape(-1, Wo.shape[2])).astype(bf),
            "ident": np.eye(128, dtype=bf),
        })
    return in_maps


_NC_CACHE = {}


def _get_nc():
    if "nc" not in _NC_CACHE:
        _NC_CACHE["nc"] = build_nc()
    return _NC_CACHE["nc"]


def run_sharded(inputs, trace=False, trace_cores=None):
    """Run the SPMD kernel; returns (full_output, BassKernelResults)."""
    x, bo = inputs["x"], inputs["bo"]
    nc = _get_nc()
    in_maps = shard_inputs(**inputs)
    res = bass_utils.run_bass_kernel_spmd(
        nc, in_maps, core_ids=list(range(N_CORES)),
        trace=trace, trace_cores=trace_cores)
    cores_per_batch = N_CORES // x.shape[0]
    out = np.empty_like(x)
    for b in range(x.shape[0]):
        acc = np.zeros((x.shape[2], x.shape[1]), np.float32)
        for c in range(b * cores_per_batch, (b + 1) * cores_per_batch):
            acc += res.results[c]["outT"]
        out[b] = acc.T + bo[None, :]
    return out, res


def kernel(**inputs):
    out, _ = run_sharded(inputs)
    return out


# revision 8
# speedup vs baseline: 1.2785x; 1.0277x over previous
"""Multi-head attention with exclusive post-processing, sharded over 8 trn2 cores.

Sharding: data-parallel over batch (2) x tensor-parallel over heads (16 -> 4/core).
Each core computes a partial transposed output [D, S] for its batch from its 4
heads; the host sums the 4 partials per batch, transposes back, and adds bo.

Per-core layout (feature-major "T" = [feature, position]); heads processed as
PAIRS stacked on the partition axis so DVE/ACT work runs at full 128-lane width:
  QT/KT/VT [128, S]  per pair (bf16)
  vprime   [128 pos, kc, h, v|ones]  position-major V with a 64-wide ones block
  scores: the two heads of a pair run as CONCURRENT K=64 matmuls on disjoint
    PE row groups (tile_position (0,0)/(64,0)) into one [128, 2*512] PSUM tile,
    so one [128,1024] exp covers both heads. ScalarE (the critical engine at
    ~147us of exp) sees the same element count as per-head processing; the PE
    sees half the score cost.
  attn@V: lhsT = [v | ones] per head -> rows 0:64 unnormalized Y, rows 64:128
    softmax denominator (free: PE matmul cost is independent of output rows).
  exclusive step, pair-packed: y_excl = (Y - (Y.v)/(sum v^2) v) / denom with
    both reciprocals on DVE (reciprocal_approx_fast, ~51 ULP) instead of
    exp(-ln(x)) on the busy ScalarE. Pair sums via one block-diagonal ones
    matmul (K=128).
  out-proj: per-pair K=128 contraction (wo stacked [128, D]).

Emission order: kc-streamed projections start as DMA chunks land; a queue of
small PE "filler" tasks (remaining projections, vprime chunks, out-proj
groups) is pumped one per kc inside the attention loops so the PE never idles
long enough for the HAM clock gate to re-throttle, and ScalarE stays fed.
"""

from contextlib import ExitStack

import ml_dtypes
import numpy as np

import concourse.mybir as mybir
import concourse.tile as tile
from concourse import bacc, bass_utils

F32 = mybir.dt.float32
BF16 = mybir.dt.bfloat16
AF = mybir.ActivationFunctionType

B, S_FULL, D_FULL, H_FULL = 2, 2048, 1024, 16
HD = 64
N_CORES = 8
HEADS_PER_CORE = H_FULL * B // N_CORES  # 4


def build_nc(S=S_FULL, D=D_FULL, HL=HEADS_PER_CORE):
    P = 128
    nH = HL * HD          # local fused head dim (256)
    KC = D // P           # x contraction chunks (8)
    NKc = S // P          # key chunks (16)
    QB = 512              # q block (one PSUM bank per head)
    NQ = S // QB          # 4
    NP = HL // 2          # head pairs (2)
    XH = S // 2           # x DMA column half

    nc = bacc.Bacc(None, target_bir_lowering=False)

    xT_d = nc.dram_tensor("xT", [D, S], BF16, kind="ExternalInput")
    wq_d = nc.dram_tensor("wq", [D, nH], BF16, kind="ExternalInput")
    wk_d = nc.dram_tensor("wk", [D, nH], BF16, kind="ExternalInput")
    wv_d = nc.dram_tensor("wv", [D, nH], BF16, kind="ExternalInput")
    wo_d = nc.dram_tensor("wo", [nH, D], BF16, kind="ExternalInput")
    id_d = nc.dram_tensor("ident", [P, P], BF16, kind="ExternalInput")
    outT_d = nc.dram_tensor("outT", [D, S], F32, kind="ExternalOutput")

    with tile.TileContext(nc) as tc, ExitStack() as ctx:
        consts = ctx.enter_context(tc.tile_pool(name="consts", bufs=1))
        psS = ctx.enter_context(tc.tile_pool(name="psS", bufs=2, space="PSUM"))
        psY = ctx.enter_context(tc.tile_pool(name="psY", bufs=2, space="PSUM"))
        psF = ctx.enter_context(tc.tile_pool(name="psF", bufs=2, space="PSUM"))
        pP = ctx.enter_context(tc.tile_pool(name="pP", bufs=5))
        ysbp = ctx.enter_context(tc.tile_pool(name="ysbp", bufs=3))
        denp = ctx.enter_context(tc.tile_pool(name="denp", bufs=3))
        betap = ctx.enter_context(tc.tile_pool(name="betap", bufs=2))
        r2p = ctx.enter_context(tc.tile_pool(name="r2p", bufs=2))
        tmpa = ctx.enter_context(tc.tile_pool(name="tmpa", bufs=2))
        tmpb = ctx.enter_context(tc.tile_pool(name="tmpb", bufs=2))
        tmpc = ctx.enter_context(tc.tile_pool(name="tmpc", bufs=2))
        ostgp = ctx.enter_context(tc.tile_pool(name="ostgp", bufs=2))

        # ---- ACT table preload: dummy exp forces the single table-set load
        # at kernel start instead of mid-attention.
        warm = consts.tile([1, 32], F32, tag="warm")
        nc.vector.memset(warm, 1.0)
        nc.scalar.activation(out=warm, in_=warm, func=AF.Exp)

        # block-diagonal ones [128,128]: one K=128 matmul sums 64-feature
        # blocks of both pair halves (result broadcast across each half)
        bdiag = consts.tile([P, P], BF16, tag="bdiag")
        nc.vector.memset(bdiag, 0.0)
        nc.vector.memset(bdiag[0:64, 0:64], 1.0)
        nc.vector.memset(bdiag[64:128, 64:128], 1.0)

        vprime = consts.tile([P, NKc, HL, 2 * HD], BF16, tag="vprime")
        nc.vector.memset(vprime[:, :, :, HD:2 * HD], 1.0)

        # ---- input staging, split across the two HW DGE queues (sync and
        # scalar) so the 6.3MB input stream runs at ~2x one queue's ~200GB/s.
        # All scalar-queue DMAs are issued at the head, before the exp stream.
        def load_w(dram, name, eng):
            tiles = []
            for kc in range(KC):
                t = consts.tile([P, nH], BF16, tag=f"w{name}{kc}", name=f"w{name}{kc}")
                eng.dma_start(out=t, in_=dram.ap()[kc * P:(kc + 1) * P, :])
                tiles.append(t)
            return tiles

        xT_sb = [consts.tile([P, S], BF16, tag=f"xT{kc}", name=f"xT{kc}") for kc in range(KC)]

        def load_x(cb, eng):
            c0 = cb * QB
            for kc in range(KC):
                eng.dma_start(out=xT_sb[kc][:, c0:c0 + QB],
                              in_=xT_d.ap()[kc * P:(kc + 1) * P, c0:c0 + QB])

        # scalar (an HWDGE engine) is deliberately NOT used for input DMA:
        # its dispatch+ring waits would delay the exp stream by ~20us.
        ident = consts.tile([P, P], BF16, tag="ident")
        nc.sync.dma_start(out=ident, in_=id_d.ap())
        wk_sb = load_w(wk_d, "k", nc.sync)
        wq_sb = load_w(wq_d, "q", nc.sync)
        load_x(1, nc.gpsimd)
        load_x(0, nc.sync)
        wv_sb = load_w(wv_d, "v", nc.sync)
        load_x(3, nc.gpsimd)
        load_x(2, nc.sync)
        wo_sb = []
        for p in range(NP):
            t = consts.tile([P, D], BF16, tag=f"wo{p}", name=f"wo{p}")
            nc.gpsimd.dma_start(out=t, in_=wo_d.ap()[p * P:(p + 1) * P, :])
            wo_sb.append(t)

        # ---- persistent feature-major tensors ----
        QT = [consts.tile([P, S], BF16, tag=f"QT{p}", name=f"QT{p}") for p in range(NP)]
        KT = [consts.tile([P, S], BF16, tag=f"KT{p}", name=f"KT{p}") for p in range(NP)]
        VT = [consts.tile([P, S], BF16, tag=f"VT{p}", name=f"VT{p}") for p in range(NP)]
        y_excl = [consts.tile([P, S], BF16, tag=f"yx{p}", name=f"yx{p}") for p in range(NP)]

        # ---- small task emitters (run as PE fillers, a ~0.5us step at a
        # time so ScalarE never waits behind a long PE filler burst) ----
        def proj_task(w_sb, dst, p, cb):
            """dst[p][:, cb*QB:(cb+1)*QB] = W_pair.T @ x chunk (K=128 x 8)."""
            ps = psF.tile([P, QB], F32, tag="f", name=f"pj{p}{cb}")
            for kc in range(KC):
                nc.tensor.matmul(
                    ps,
                    lhsT=w_sb[kc][:, p * P:(p + 1) * P],
                    rhs=xT_sb[kc][:, cb * QB:(cb + 1) * QB],
                    start=(kc == 0), stop=(kc == KC - 1))
                if kc % 2 == 1 and kc < KC - 1:
                    yield
            nc.vector.tensor_copy(out=dst[p][:, cb * QB:(cb + 1) * QB], in_=ps)

        def vprime_task(p, kc):
            """position-major V chunk via PE transpose of the feature-major
            VT block -- ~0.3us instead of an 8-matmul projection."""
            require(("V", p, kc // (NKc // NQ)))
            ps = psF.tile([P, P], BF16, tag="f", name=f"tr{p}{kc}")
            nc.tensor.transpose(ps, VT[p][:, kc * P:(kc + 1) * P], ident)
            nc.vector.tensor_copy(out=vprime[:, kc, 2 * p, 0:HD], in_=ps[:, 0:HD])
            nc.vector.tensor_copy(out=vprime[:, kc, 2 * p + 1, 0:HD],
                                  in_=ps[:, HD:2 * HD])
            return
            yield

        def e_task(qb, mt):
            """out-proj m-tile: K=128 per pair, accumulated over both pairs."""
            q0 = qb * QB
            if False:
                yield
            ps = psF.tile([P, QB], F32, tag="f", name=f"e{qb}{mt}")
            for p in range(NP):
                nc.tensor.matmul(
                    ps,
                    lhsT=wo_sb[p][:, mt * P:(mt + 1) * P],
                    rhs=y_excl[p][:, q0:q0 + QB],
                    start=(p == 0), stop=(p == NP - 1))
            ostg = ostgp.tile([P, QB], F32, tag="ostg", name="ostg")
            nc.any.tensor_copy(out=ostg, in_=ps)
            nc.sync.dma_start(
                out=outT_d.ap()[mt * P:(mt + 1) * P, q0:q0 + QB], in_=ostg)

        # filler task queue: generator tasks are advanced one ~0.5us step at
        # a time (pump, once per kc inside attention loops) or run to
        # completion on demand (require, when a consumer is about to be
        # emitted) -- emission order defines the dependencies Tile sees, so a
        # consumer must never precede its producer task.
        fillers = []
        tasks = {}

        def pump(n=1):
            for _ in range(n):
                while fillers:
                    key = fillers[0]
                    g = tasks.get(key)
                    if g is None:
                        fillers.pop(0)
                        continue
                    try:
                        next(g)
                    except StopIteration:
                        tasks.pop(key, None)
                        fillers.pop(0)
                    break

        def require(key):
            g = tasks.pop(key, None)
            if g is not None:
                for _ in g:
                    pass

        def add_task(key, gen_fn):
            tasks[key] = gen_fn()
            fillers.append(key)

        def run_task(gen_fn):
            for _ in gen_fn():
                pass

        # ---- D1: scores + exp + attn@V for one (q-block, pair) ----
        def emit_d1(qb, p, pending=None):
            """pending = deferred exclusive chain (qb', p', ysb, den) from the
            previous block, emitted a few kc in so its serial DVE chain and
            ones-matmuls overlap this block's scores instead of head-blocking
            the in-order PE queue at the boundary."""
            q0 = qb * QB
            KTp, QTp = KT[p], QT[p]
            yp = [psY.tile([HD * 2, QB], F32, tag="y", name=f"yp{p}{h}")
                  for h in range(2)]

            def attn_v(pT, kc):
                for h in range(2):
                    nc.tensor.matmul(
                        yp[h],
                        lhsT=vprime[:, kc, 2 * p + h, :],
                        rhs=pT[:, h * QB:(h + 1) * QB],
                        start=(kc == 0), stop=(kc == NKc - 1))

            require(("Q", p, qb))
            prev = None
            for kc in range(NKc):
                require(("K", p, kc // (NKc // NQ)))
                require(("vp", p, kc))
                sc = psS.tile([P, 2 * QB], F32, tag="s", name=f"sc{p}")
                # the two heads' K=64 score matmuls run concurrently on
                # disjoint PE row groups (lhsT/rhs at base 0 vs 64)
                for h in range(2):
                    nc.tensor.matmul(
                        sc[:, h * QB:(h + 1) * QB],
                        lhsT=KTp[h * HD:(h + 1) * HD, kc * P:(kc + 1) * P],
                        rhs=QTp[h * HD:(h + 1) * HD, q0:q0 + QB],
                        start=True, stop=True)
                pT = pP.tile([P, 2 * QB], BF16, tag="pt", name=f"pt{p}")
                nc.scalar.activation(out=pT, in_=sc, func=AF.Exp, scale=0.125)
                if prev is not None:
                    attn_v(*prev)
                prev = (pT, kc)
                if kc == 3 and pending is not None:
                    emit_d2(*pending)
                else:
                    pump(1)
            attn_v(*prev)

            ysb = ysbp.tile([P, QB], BF16, tag="ysb", name="ysb")
            den = denp.tile([P, QB], F32, tag="den", name="den")
            nc.vector.tensor_copy(out=ysb[0:64, :], in_=yp[0][0:64, :])
            nc.vector.tensor_copy(out=ysb[64:128, :], in_=yp[1][0:64, :])
            nc.vector.tensor_copy(out=den[0:64, :], in_=yp[0][64:128, :])
            nc.vector.tensor_copy(out=den[64:128, :], in_=yp[1][64:128, :])
            return ysb, den

        # ---- D2: pair-packed exclusive step ----
        def emit_d2(qb, p, ysb, den):
            require(("V", p, qb))
            q0 = qb * QB
            vtp = VT[p][:, q0:q0 + QB]
            beta = betap.tile([P, QB], F32, tag="beta", name="beta")
            nc.vector.reciprocal_approx_fast(out=beta, in_=den)
            vsq = tmpa.tile([P, QB], BF16, tag="vsq", name="vsq")
            nc.vector.tensor_mul(vsq, vtp, vtp)
            d2B = psF.tile([P, QB], F32, tag="f", name="d2B")
            nc.tensor.matmul(d2B, lhsT=bdiag, rhs=vsq, start=True, stop=True)
            r2 = r2p.tile([P, QB], F32, tag="r2", name="r2")
            nc.vector.reciprocal_approx_fast(out=r2, in_=d2B)
            t_yv = tmpa.tile([P, QB], BF16, tag="tyv", name="tyv")
            nc.vector.tensor_mul(t_yv, ysb, vtp)
            d1B = psF.tile([P, QB], F32, tag="f", name="d1B")
            nc.tensor.matmul(d1B, lhsT=bdiag, rhs=t_yv, start=True, stop=True)
            aB = tmpb.tile([P, QB], BF16, tag="ab", name="ab")
            nc.vector.tensor_mul(aB, d1B, r2)
            t2 = tmpc.tile([P, QB], BF16, tag="t2", name="t2")
            nc.vector.tensor_mul(t2, vtp, aB)
            u = tmpb.tile([P, QB], BF16, tag="u", name="u")
            nc.vector.tensor_sub(u, ysb, t2)
            nc.vector.tensor_mul(y_excl[p][:, q0:q0 + QB], u, beta)

        # ---- emission ----
        # upfront: keys/queries/values for pair0 q-block 0 and the first
        # vprime chunks; everything else queues as fillers (requires enforce
        # dependency order regardless of pump progress).
        run_task(lambda: proj_task(wk_sb, KT, 0, 0))
        run_task(lambda: proj_task(wq_sb, QT, 0, 0))
        run_task(lambda: proj_task(wv_sb, VT, 0, 0))
        run_task(lambda: vprime_task(0, 0))
        run_task(lambda: vprime_task(0, 1))
        for kc in range(2, 4):
            add_task(("vp", 0, kc), lambda kc=kc: vprime_task(0, kc))
        add_task(("K", 0, 1), lambda: proj_task(wk_sb, KT, 0, 1))
        for kc in range(4, 8):
            add_task(("vp", 0, kc), lambda kc=kc: vprime_task(0, kc))
        add_task(("K", 0, 2), lambda: proj_task(wk_sb, KT, 0, 2))
        add_task(("V", 0, 2), lambda: proj_task(wv_sb, VT, 0, 2))
        for kc in range(8, 12):
            add_task(("vp", 0, kc), lambda kc=kc: vprime_task(0, kc))
        add_task(("K", 0, 3), lambda: proj_task(wk_sb, KT, 0, 3))
        add_task(("V", 0, 3), lambda: proj_task(wv_sb, VT, 0, 3))
        for kc in range(12, NKc):
            add_task(("vp", 0, kc), lambda kc=kc: vprime_task(0, kc))
        add_task(("V", 0, 1), lambda: proj_task(wv_sb, VT, 0, 1))
        for cb in range(1, NQ):
            add_task(("Q", 0, cb), lambda cb=cb: proj_task(wq_sb, QT, 0, cb))
        for cb in range(NQ):
            add_task(("K", 1, cb), lambda cb=cb: proj_task(wk_sb, KT, 1, cb))
            add_task(("V", 1, cb), lambda cb=cb: proj_task(wv_sb, VT, 1, cb))
        for kc in range(NKc):
            add_task(("vp", 1, kc), lambda kc=kc: vprime_task(1, kc))
        for cb in range(NQ):
            add_task(("Q", 1, cb), lambda cb=cb: proj_task(wq_sb, QT, 1, cb))

        # pair 0 over all q-blocks, then pair 1; each block's exclusive chain
        # is deferred into the next block's kc loop; out-proj groups are
        # appended as fillers once both pairs of a q-block are done.
        pending = None
        for p in range(NP):
            for qb in range(NQ):
                ysb, den = emit_d1(qb, p, pending)
                if pending is not None and pending[1] == NP - 1:
                    for mt in range(D // P):
                        add_task(("e", pending[0], mt),
                                 lambda qb=pending[0], mt=mt: e_task(qb, mt))
                pending = (qb, p, ysb, den)
        emit_d2(*pending)
        # drain remaining fillers, then the last q-block's out-proj
        while fillers:
            pump(1)
        for mt in range(D // P):
            run_task(lambda mt=mt: e_task(NQ - 1, mt))

    nc.finalize()
    return nc


def shard_inputs(x, Wq, bq, Wk, bk, Wv, bv, Wo, bo, n_cores=N_CORES):
    """Full inputs -> per-core input maps (host-side transpose/slice/reshape)."""
    assert not (np.any(bq) or np.any(bk) or np.any(bv)), "nonzero qkv bias unsupported"
    H = Wq.shape[1]
    cores_per_batch = n_cores // x.shape[0]
    hl = H // cores_per_batch
    bf = ml_dtypes.bfloat16
    in_maps = []
    for c in range(n_cores):
        b = c // cores_per_batch
        h0 = (c % cores_per_batch) * hl
        in_maps.append({
            "xT": np.ascontiguousarray(x[b].T).astype(bf),
            "wq": np.ascontiguousarray(Wq[:, h0:h0 + hl, :].reshape(Wq.shape[0], -1)).astype(bf),
            "wk": np.ascontiguousarray(Wk[:, h0:h0 + hl, :].reshape(Wk.shape[0], -1)).astype(bf),
            "wv": np.ascontiguousarray(Wv[:, h0:h0 + hl, :].reshape(Wv.shape[0], -1)).astype(bf),
            "wo": np.ascontiguousarray(Wo[h0:h0 + hl].reshape(-1, Wo.shape[2])).astype(bf),
            "ident": np.eye(128, dtype=bf),
        })
    return in_maps


_NC_CACHE = {}


def _get_nc():
    if "nc" not in _NC_CACHE:
        _NC_CACHE["nc"] = build_nc()
    return _NC_CACHE["nc"]


def run_sharded(inputs, trace=False, trace_cores=None):
    """Run the SPMD kernel; returns (full_output, BassKernelResults)."""
    x, bo = inputs["x"], inputs["bo"]
    nc = _get_nc()
    in_maps = shard_inputs(**inputs)
    res = bass_utils.run_bass_kernel_spmd(
        nc, in_maps, core_ids=list(range(N_CORES)),
        trace=trace, trace_cores=trace_cores)
    cores_per_batch = N_CORES // x.shape[0]
    out = np.empty_like(x)
    for b in range(x.shape[0]):
        acc = np.zeros((x.shape[2], x.shape[1]), np.float32)
        for c in range(b * cores_per_batch, (b + 1) * cores_per_batch):
            acc += res.results[c]["outT"]
        out[b] = acc.T + bo[None, :]
    return out, res


def kernel(**inputs):
    out, _ = run_sharded(inputs)
    return out


# revision 9
# speedup vs baseline: 1.3203x; 1.0327x over previous
"""Multi-head attention with exclusive post-processing, sharded over 8 trn2 cores.

Sharding: data-parallel over batch (2) x tensor-parallel over heads (16 -> 4/core).
Each core computes a partial transposed output [D, S] for its batch from its 4
heads; the host sums the 4 partials per batch, transposes back, and adds bo.

Per-core layout (feature-major "T" = [feature, position]); heads processed as
PAIRS stacked on the partition axis so DVE/ACT work runs at full 128-lane width:
  QT/KT/VT [128, S]  per pair (bf16)
  vprime   [128 pos, kc, h, v|ones]  position-major V with a 64-wide ones block
  scores: the two heads of a pair run as CONCURRENT K=64 matmuls on disjoint
    PE row groups (tile_position (0,0)/(64,0)) into one [128, 2*512] PSUM tile,
    so one [128,1024] exp covers both heads. ScalarE (the critical engine at
    ~147us of exp) sees the same element count as per-head processing; the PE
    sees half the score cost.
  attn@V: lhsT = [v | ones] per head -> rows 0:64 unnormalized Y, rows 64:128
    softmax denominator (free: PE matmul cost is independent of output rows).
  exclusive step, pair-packed: y_excl = (Y - (Y.v)/(sum v^2) v) / denom with
    both reciprocals on DVE (reciprocal_approx_fast, ~51 ULP) instead of
    exp(-ln(x)) on the busy ScalarE. Pair sums via one block-diagonal ones
    matmul (K=128).
  out-proj: per-pair K=128 contraction (wo stacked [128, D]).

Emission order: kc-streamed projections start as DMA chunks land; a queue of
small PE "filler" tasks (remaining projections, vprime chunks, out-proj
groups) is pumped one per kc inside the attention loops so the PE never idles
long enough for the HAM clock gate to re-throttle, and ScalarE stays fed.
"""

from contextlib import ExitStack

import ml_dtypes
import numpy as np

import concourse.mybir as mybir
import concourse.tile as tile
from concourse import bacc, bass_utils

F32 = mybir.dt.float32
BF16 = mybir.dt.bfloat16
AF = mybir.ActivationFunctionType

B, S_FULL, D_FULL, H_FULL = 2, 2048, 1024, 16
HD = 64
N_CORES = 8
HEADS_PER_CORE = H_FULL * B // N_CORES  # 4


def build_nc(S=S_FULL, D=D_FULL, HL=HEADS_PER_CORE):
    P = 128
    nH = HL * HD          # local fused head dim (256)
    KC = D // P           # x contraction chunks (8)
    NKc = S // P          # key chunks (16)
    QB = 512              # q block (one PSUM bank per head)
    NQ = S // QB          # 4
    NP = HL // 2          # head pairs (2)
    XH = S // 2           # x DMA column half

    nc = bacc.Bacc(None, target_bir_lowering=False)

    xT_d = nc.dram_tensor("xT", [D, S], BF16, kind="ExternalInput")
    wq_d = nc.dram_tensor("wq", [D, nH], BF16, kind="ExternalInput")
    wk_d = nc.dram_tensor("wk", [D, nH], BF16, kind="ExternalInput")
    wv_d = nc.dram_tensor("wv", [D, nH], BF16, kind="ExternalInput")
    wo_d = nc.dram_tensor("wo", [nH, D], BF16, kind="ExternalInput")
    id_d = nc.dram_tensor("ident", [P, P], BF16, kind="ExternalInput")
    outT_d = nc.dram_tensor("outT", [D, S], F32, kind="ExternalOutput")

    with tile.TileContext(nc) as tc, ExitStack() as ctx:
        consts = ctx.enter_context(tc.tile_pool(name="consts", bufs=1))
        psS = ctx.enter_context(tc.tile_pool(name="psS", bufs=2, space="PSUM"))
        psY = ctx.enter_context(tc.tile_pool(name="psY", bufs=2, space="PSUM"))
        psF = ctx.enter_context(tc.tile_pool(name="psF", bufs=2, space="PSUM"))
        pP = ctx.enter_context(tc.tile_pool(name="pP", bufs=5))
        ysbp = ctx.enter_context(tc.tile_pool(name="ysbp", bufs=3))
        denp = ctx.enter_context(tc.tile_pool(name="denp", bufs=3))
        betap = ctx.enter_context(tc.tile_pool(name="betap", bufs=2))
        r2p = ctx.enter_context(tc.tile_pool(name="r2p", bufs=2))
        tmpa = ctx.enter_context(tc.tile_pool(name="tmpa", bufs=2))
        tmpb = ctx.enter_context(tc.tile_pool(name="tmpb", bufs=2))
        tmpc = ctx.enter_context(tc.tile_pool(name="tmpc", bufs=2))
        ostgp = ctx.enter_context(tc.tile_pool(name="ostgp", bufs=2))

        # ---- ACT table preload: dummy exp forces the single table-set load
        # at kernel start instead of mid-attention.
        warm = consts.tile([1, 32], F32, tag="warm")
        nc.vector.memset(warm, 1.0)
        nc.scalar.activation(out=warm, in_=warm, func=AF.Exp)

        # block-diagonal ones [128,128]: one K=128 matmul sums 64-feature
        # blocks of both pair halves (result broadcast across each half)
        bdiag = consts.tile([P, P], BF16, tag="bdiag")
        nc.vector.memset(bdiag, 0.0)
        nc.vector.memset(bdiag[0:64, 0:64], 1.0)
        nc.vector.memset(bdiag[64:128, 64:128], 1.0)

        vprime = consts.tile([P, NKc, HL, 2 * HD], BF16, tag="vprime")
        nc.vector.memset(vprime[:, :, :, HD:2 * HD], 1.0)

        # ---- input staging, split across the two HW DGE queues (sync and
        # scalar) so the 6.3MB input stream runs at ~2x one queue's ~200GB/s.
        # All scalar-queue DMAs are issued at the head, before the exp stream.
        def load_w(dram, name, eng):
            tiles = []
            for kc in range(KC):
                t = consts.tile([P, nH], BF16, tag=f"w{name}{kc}", name=f"w{name}{kc}")
                eng.dma_start(out=t, in_=dram.ap()[kc * P:(kc + 1) * P, :])
                tiles.append(t)
            return tiles

        xT_sb = [consts.tile([P, S], BF16, tag=f"xT{kc}", name=f"xT{kc}") for kc in range(KC)]

        def load_x(cb, eng):
            c0 = cb * QB
            for kc in range(KC):
                eng.dma_start(out=xT_sb[kc][:, c0:c0 + QB],
                              in_=xT_d.ap()[kc * P:(kc + 1) * P, c0:c0 + QB])

        # scalar (an HWDGE engine) is deliberately NOT used for input DMA:
        # its dispatch+ring waits would delay the exp stream by ~20us.
        ident = consts.tile([P, P], BF16, tag="ident")
        nc.sync.dma_start(out=ident, in_=id_d.ap())
        wk_sb = load_w(wk_d, "k", nc.sync)
        wq_sb = load_w(wq_d, "q", nc.gpsimd)
        load_x(0, nc.sync)
        load_x(1, nc.gpsimd)
        wv_sb = load_w(wv_d, "v", nc.sync)
        load_x(3, nc.gpsimd)
        load_x(2, nc.sync)
        wo_sb = []
        for p in range(NP):
            t = consts.tile([P, D], BF16, tag=f"wo{p}", name=f"wo{p}")
            nc.gpsimd.dma_start(out=t, in_=wo_d.ap()[p * P:(p + 1) * P, :])
            wo_sb.append(t)

        # ---- persistent feature-major tensors ----
        QT = [consts.tile([P, S], BF16, tag=f"QT{p}", name=f"QT{p}") for p in range(NP)]
        KT = [consts.tile([P, S], BF16, tag=f"KT{p}", name=f"KT{p}") for p in range(NP)]
        VT = [consts.tile([P, S], BF16, tag=f"VT{p}", name=f"VT{p}") for p in range(NP)]
        y_excl = [consts.tile([P, S], BF16, tag=f"yx{p}", name=f"yx{p}") for p in range(NP)]

        # ---- small task emitters (run as PE fillers, a ~0.5us step at a
        # time so ScalarE never waits behind a long PE filler burst) ----
        def proj_task(w_sb, dst, p, cb):
            """dst[p][:, cb*QB:(cb+1)*QB] = W_pair.T @ x chunk (K=128 x 8)."""
            ps = psF.tile([P, QB], F32, tag="f", name=f"pj{p}{cb}")
            for kc in range(KC):
                nc.tensor.matmul(
                    ps,
                    lhsT=w_sb[kc][:, p * P:(p + 1) * P],
                    rhs=xT_sb[kc][:, cb * QB:(cb + 1) * QB],
                    start=(kc == 0), stop=(kc == KC - 1))
                if kc % 2 == 1 and kc < KC - 1:
                    yield
            nc.vector.tensor_copy(out=dst[p][:, cb * QB:(cb + 1) * QB], in_=ps)

        def vprime_task(p, kc):
            """position-major V chunk via PE transpose of the feature-major
            VT block -- ~0.3us instead of an 8-matmul projection."""
            require(("V", p, kc // (NKc // NQ)))
            ps = psF.tile([P, P], BF16, tag="f", name=f"tr{p}{kc}")
            nc.tensor.transpose(ps, VT[p][:, kc * P:(kc + 1) * P], ident)
            nc.vector.tensor_copy(out=vprime[:, kc, 2 * p, 0:HD], in_=ps[:, 0:HD])
            nc.vector.tensor_copy(out=vprime[:, kc, 2 * p + 1, 0:HD],
                                  in_=ps[:, HD:2 * HD])
            return
            yield

        def e_task(qb, mt):
            """out-proj m-tile: K=128 per pair, accumulated over both pairs."""
            q0 = qb * QB
            if False:
                yield
            ps = psF.tile([P, QB], F32, tag="f", name=f"e{qb}{mt}")
            for p in range(NP):
                nc.tensor.matmul(
                    ps,
                    lhsT=wo_sb[p][:, mt * P:(mt + 1) * P],
                    rhs=y_excl[p][:, q0:q0 + QB],
                    start=(p == 0), stop=(p == NP - 1))
            ostg = ostgp.tile([P, QB], F32, tag="ostg", name="ostg")
            nc.vector.tensor_copy(out=ostg, in_=ps)
            nc.sync.dma_start(
                out=outT_d.ap()[mt * P:(mt + 1) * P, q0:q0 + QB], in_=ostg)

        # filler task queue: generator tasks are advanced one ~0.5us step at
        # a time (pump, once per kc inside attention loops) or run to
        # completion on demand (require, when a consumer is about to be
        # emitted) -- emission order defines the dependencies Tile sees, so a
        # consumer must never precede its producer task.
        fillers = []
        tasks = {}

        def pump(n=1):
            for _ in range(n):
                while fillers:
                    key = fillers[0]
                    g = tasks.get(key)
                    if g is None:
                        fillers.pop(0)
                        continue
                    try:
                        next(g)
                    except StopIteration:
                        tasks.pop(key, None)
                        fillers.pop(0)
                    break

        def require(key):
            g = tasks.pop(key, None)
            if g is not None:
                for _ in g:
                    pass

        def add_task(key, gen_fn):
            tasks[key] = gen_fn()
            fillers.append(key)

        def run_task(gen_fn):
            for _ in gen_fn():
                pass

        # ---- D1: scores + exp + attn@V for one (q-block, pair) ----
        def emit_d1(qb, p, pending=None):
            """pending = deferred exclusive chain (qb', p', ysb, den) from the
            previous block, emitted a few kc in so its serial DVE chain and
            ones-matmuls overlap this block's scores instead of head-blocking
            the in-order PE queue at the boundary."""
            q0 = qb * QB
            KTp, QTp = KT[p], QT[p]
            yp = [psY.tile([HD * 2, QB], F32, tag="y", name=f"yp{p}{h}")
                  for h in range(2)]

            def attn_v(pT, kc):
                for h in range(2):
                    nc.tensor.matmul(
                        yp[h],
                        lhsT=vprime[:, kc, 2 * p + h, :],
                        rhs=pT[:, h * QB:(h + 1) * QB],
                        start=(kc == 0), stop=(kc == NKc - 1))

            require(("Q", p, qb))
            # prefetch the next block's projections mid-loop so its first
            # scores are never blocked on a cold 8-matmul require burst
            if qb + 1 < NQ:
                prefetch = [("Q", p, qb + 1)]
            elif p + 1 < NP:
                prefetch = [("K", p + 1, 0), ("K", p + 1, 1), ("K", p + 1, 2),
                            ("K", p + 1, 3), ("V", p + 1, 0), ("Q", p + 1, 0),
                            ("vp", p + 1, 0), ("vp", p + 1, 1)]
            else:
                prefetch = []
            prev = None
            for kc in range(NKc):
                require(("K", p, kc // (NKc // NQ)))
                require(("vp", p, kc))
                if kc >= 7 and prefetch:
                    require(prefetch.pop(0))
                sc = psS.tile([P, 2 * QB], F32, tag="s", name=f"sc{p}")
                # the two heads' K=64 score matmuls run concurrently on
                # disjoint PE row groups (lhsT/rhs at base 0 vs 64)
                for h in range(2):
                    nc.tensor.matmul(
                        sc[:, h * QB:(h + 1) * QB],
                        lhsT=KTp[h * HD:(h + 1) * HD, kc * P:(kc + 1) * P],
                        rhs=QTp[h * HD:(h + 1) * HD, q0:q0 + QB],
                        start=True, stop=True)
                pT = pP.tile([P, 2 * QB], BF16, tag="pt", name=f"pt{p}")
                nc.scalar.activation(out=pT, in_=sc, func=AF.Exp, scale=0.125)
                if prev is not None:
                    attn_v(*prev)
                prev = (pT, kc)
                if kc == 3 and pending is not None:
                    emit_d2(*pending)
                else:
                    pump(1)
            attn_v(*prev)

            ysb = ysbp.tile([P, QB], BF16, tag="ysb", name="ysb")
            den = denp.tile([P, QB], F32, tag="den", name="den")
            nc.vector.tensor_copy(out=ysb[0:64, :], in_=yp[0][0:64, :])
            nc.vector.tensor_copy(out=ysb[64:128, :], in_=yp[1][0:64, :])
            nc.vector.tensor_copy(out=den[0:64, :], in_=yp[0][64:128, :])
            nc.vector.tensor_copy(out=den[64:128, :], in_=yp[1][64:128, :])
            return ysb, den

        # ---- D2: pair-packed exclusive step ----
        def emit_d2(qb, p, ysb, den):
            require(("V", p, qb))
            q0 = qb * QB
            vtp = VT[p][:, q0:q0 + QB]
            beta = betap.tile([P, QB], F32, tag="beta", name="beta")
            nc.vector.reciprocal_approx_fast(out=beta, in_=den)
            vsq = tmpa.tile([P, QB], BF16, tag="vsq", name="vsq")
            nc.vector.tensor_mul(vsq, vtp, vtp)
            d2B = psF.tile([P, QB], F32, tag="f", name="d2B")
            nc.tensor.matmul(d2B, lhsT=bdiag, rhs=vsq, start=True, stop=True)
            r2 = r2p.tile([P, QB], F32, tag="r2", name="r2")
            nc.vector.reciprocal_approx_fast(out=r2, in_=d2B)
            t_yv = tmpa.tile([P, QB], BF16, tag="tyv", name="tyv")
            nc.vector.tensor_mul(t_yv, ysb, vtp)
            d1B = psF.tile([P, QB], F32, tag="f", name="d1B")
            nc.tensor.matmul(d1B, lhsT=bdiag, rhs=t_yv, start=True, stop=True)
            aB = tmpb.tile([P, QB], BF16, tag="ab", name="ab")
            nc.vector.tensor_mul(aB, d1B, r2)
            t2 = tmpc.tile([P, QB], BF16, tag="t2", name="t2")
            nc.vector.tensor_mul(t2, vtp, aB)
            u = tmpb.tile([P, QB], BF16, tag="u", name="u")
            nc.vector.tensor_sub(u, ysb, t2)
            nc.vector.tensor_mul(y_excl[p][:, q0:q0 + QB], u, beta)

        # ---- emission ----
        # upfront: keys/queries/values for pair0 q-block 0 and the first
        # vprime chunks; everything else queues as fillers (requires enforce
        # dependency order regardless of pump progress).
        run_task(lambda: proj_task(wk_sb, KT, 0, 0))
        run_task(lambda: proj_task(wq_sb, QT, 0, 0))
        run_task(lambda: proj_task(wv_sb, VT, 0, 0))
        run_task(lambda: vprime_task(0, 0))
        run_task(lambda: vprime_task(0, 1))
        for kc in range(2, 4):
            add_task(("vp", 0, kc), lambda kc=kc: vprime_task(0, kc))
        add_task(("K", 0, 1), lambda: proj_task(wk_sb, KT, 0, 1))
        for kc in range(4, 8):
            add_task(("vp", 0, kc), lambda kc=kc: vprime_task(0, kc))
        add_task(("K", 0, 2), lambda: proj_task(wk_sb, KT, 0, 2))
        add_task(("V", 0, 2), lambda: proj_task(wv_sb, VT, 0, 2))
        for kc in range(8, 12):
            add_task(("vp", 0, kc), lambda kc=kc: vprime_task(0, kc))
        add_task(("K", 0, 3), lambda: proj_task(wk_sb, KT, 0, 3))
        add_task(("V", 0, 3), lambda: proj_task(wv_sb, VT, 0, 3))
        for kc in range(12, NKc):
            add_task(("vp", 0, kc), lambda kc=kc: vprime_task(0, kc))
        add_task(("V", 0, 1), lambda: proj_task(wv_sb, VT, 0, 1))
        for cb in range(1, NQ):
            add_task(("Q", 0, cb), lambda cb=cb: proj_task(wq_sb, QT, 0, cb))
        for cb in range(NQ):
            add_task(("K", 1, cb), lambda cb=cb: proj_task(wk_sb, KT, 1, cb))
            add_task(("V", 1, cb), lambda cb=cb: proj_task(wv_sb, VT, 1, cb))
        for kc in range(NKc):
            add_task(("vp", 1, kc), lambda kc=kc: vprime_task(1, kc))
        for cb in range(NQ):
            add_task(("Q", 1, cb), lambda cb=cb: proj_task(wq_sb, QT, 1, cb))

        # pair 0 over all q-blocks, then pair 1; each block's exclusive chain
        # is deferred into the next block's kc loop; out-proj groups are
        # appended as fillers once both pairs of a q-block are done.
        pending = None
        for p in range(NP):
            for qb in range(NQ):
                ysb, den = emit_d1(qb, p, pending)
                if pending is not None and pending[1] == NP - 1:
                    for mt in range(D // P):
                        add_task(("e", pending[0], mt),
                                 lambda qb=pending[0], mt=mt: e_task(qb, mt))
                pending = (qb, p, ysb, den)
        emit_d2(*pending)
        # drain remaining fillers, then the last q-block's out-proj
        while fillers:
            pump(1)
        for mt in range(D // P):
            run_task(lambda mt=mt: e_task(NQ - 1, mt))

    nc.finalize()
    return nc


def shard_inputs(x, Wq, bq, Wk, bk, Wv, bv, Wo, bo, n_cores=N_CORES):
    """Full inputs -> per-core input maps (host-side transpose/slice/reshape)."""
    assert not (np.any(bq) or np.any(bk) or np.any(bv)), "nonzero qkv bias unsupported"
    H = Wq.shape[1]
    cores_per_batch = n_cores // x.shape[0]
    hl = H // cores_per_batch
    bf = ml_dtypes.bfloat16
    in_maps = []
    for c in range(n_cores):
        b = c // cores_per_batch
        h0 = (c % cores_per_batch) * hl
        in_maps.append({
            "xT": np.ascontiguousarray(x[b].T).astype(bf),
            "wq": np.ascontiguousarray(Wq[:, h0:h0 + hl, :].reshape(Wq.shape[0], -1)).astype(bf),
            "wk": np.ascontiguousarray(Wk[:, h0:h0 + hl, :].reshape(Wk.shape[0], -1)).astype(bf),
            "wv": np.ascontiguousarray(Wv[:, h0:h0 + hl, :].reshape(Wv.shape[0], -1)).astype(bf),
            "wo": np.ascontiguousarray(Wo[h0:h0 + hl].reshape(-1, Wo.shape[2])).astype(bf),
            "ident": np.eye(128, dtype=bf),
        })
    return in_maps


_NC_CACHE = {}


def _get_nc():
    if "nc" not in _NC_CACHE:
        _NC_CACHE["nc"] = build_nc()
    return _NC_CACHE["nc"]


def run_sharded(inputs, trace=False, trace_cores=None):
    """Run the SPMD kernel; returns (full_output, BassKernelResults)."""
    x, bo = inputs["x"], inputs["bo"]
    nc = _get_nc()
    in_maps = shard_inputs(**inputs)
    res = bass_utils.run_bass_kernel_spmd(
        nc, in_maps, core_ids=list(range(N_CORES)),
        trace=trace, trace_cores=trace_cores)
    cores_per_batch = N_CORES // x.shape[0]
    out = np.empty_like(x)
    for b in range(x.shape[0]):
        acc = np.zeros((x.shape[2], x.shape[1]), np.float32)
        for c in range(b * cores_per_batch, (b + 1) * cores_per_batch):
            acc += res.results[c]["outT"]
        out[b] = acc.T + bo[None, :]
    return out, res


def kernel(**inputs):
    out, _ = run_sharded(inputs)
    return out


# revision 10
# speedup vs baseline: 1.3400x; 1.0149x over previous
"""Multi-head attention with exclusive post-processing, sharded over 8 trn2 cores.

Sharding: data-parallel over batch (2) x tensor-parallel over heads (16 -> 4/core).
Each core computes a partial transposed output [D, S] for its batch from its 4
heads; the host sums the 4 partials per batch, transposes back, and adds bo.

Per-core layout (feature-major "T" = [feature, position]); heads processed as
PAIRS stacked on the partition axis so DVE/ACT work runs at full 128-lane width:
  QT/KT/VT [128, S]  per pair (bf16)
  vprime   [128 pos, kc, h, v|ones]  position-major V with a 64-wide ones block
  scores: the two heads of a pair run as CONCURRENT K=64 matmuls on disjoint
    PE row groups (tile_position (0,0)/(64,0)) into one [128, 2*512] PSUM tile,
    so one [128,1024] exp covers both heads. ScalarE (the critical engine at
    ~147us of exp) sees the same element count as per-head processing; the PE
    sees half the score cost.
  attn@V: lhsT = [v | ones] per head -> rows 0:64 unnormalized Y, rows 64:128
    softmax denominator (free: PE matmul cost is independent of output rows).
  exclusive step, pair-packed: y_excl = (Y - (Y.v)/(sum v^2) v) / denom with
    both reciprocals on DVE (reciprocal_approx_fast, ~51 ULP) instead of
    exp(-ln(x)) on the busy ScalarE. Pair sums via one block-diagonal ones
    matmul (K=128).
  out-proj: per-pair K=128 contraction (wo stacked [128, D]).

Emission order: kc-streamed projections start as DMA chunks land; a queue of
small PE "filler" tasks (remaining projections, vprime chunks, out-proj
groups) is pumped one per kc inside the attention loops so the PE never idles
long enough for the HAM clock gate to re-throttle, and ScalarE stays fed.
"""

from contextlib import ExitStack

import ml_dtypes
import numpy as np

import concourse.mybir as mybir
import concourse.tile as tile
from concourse import bacc, bass_utils

F32 = mybir.dt.float32
BF16 = mybir.dt.bfloat16
AF = mybir.ActivationFunctionType

B, S_FULL, D_FULL, H_FULL = 2, 2048, 1024, 16
HD = 64
N_CORES = 8
HEADS_PER_CORE = H_FULL * B // N_CORES  # 4


def build_nc(S=S_FULL, D=D_FULL, HL=HEADS_PER_CORE):
    P = 128
    nH = HL * HD          # local fused head dim (256)
    KC = D // P           # x contraction chunks (8)
    NKc = S // P          # key chunks (16)
    QB = 512              # q block (one PSUM bank per head)
    NQ = S // QB          # 4
    NP = HL // 2          # head pairs (2)
    XH = S // 2           # x DMA column half

    nc = bacc.Bacc(None, target_bir_lowering=False)

    xT_d = nc.dram_tensor("xT", [D, S], BF16, kind="ExternalInput")
    wq_d = nc.dram_tensor("wq", [D, nH], BF16, kind="ExternalInput")
    wk_d = nc.dram_tensor("wk", [D, nH], BF16, kind="ExternalInput")
    wv_d = nc.dram_tensor("wv", [D, nH], BF16, kind="ExternalInput")
    wo_d = nc.dram_tensor("wo", [nH, D], BF16, kind="ExternalInput")
    id_d = nc.dram_tensor("ident", [P, P], BF16, kind="ExternalInput")
    outT_d = nc.dram_tensor("outT", [D, S], F32, kind="ExternalOutput")

    with tile.TileContext(nc) as tc, ExitStack() as ctx:
        consts = ctx.enter_context(tc.tile_pool(name="consts", bufs=1))
        psS = ctx.enter_context(tc.tile_pool(name="psS", bufs=2, space="PSUM"))
        psY = ctx.enter_context(tc.tile_pool(name="psY", bufs=2, space="PSUM"))
        psF = ctx.enter_context(tc.tile_pool(name="psF", bufs=2, space="PSUM"))
        pP = ctx.enter_context(tc.tile_pool(name="pP", bufs=5))
        ysbp = ctx.enter_context(tc.tile_pool(name="ysbp", bufs=3))
        denp = ctx.enter_context(tc.tile_pool(name="denp", bufs=3))
        betap = ctx.enter_context(tc.tile_pool(name="betap", bufs=2))
        r2p = ctx.enter_context(tc.tile_pool(name="r2p", bufs=2))
        tmpa = ctx.enter_context(tc.tile_pool(name="tmpa", bufs=2))
        tmpb = ctx.enter_context(tc.tile_pool(name="tmpb", bufs=2))
        tmpc = ctx.enter_context(tc.tile_pool(name="tmpc", bufs=2))
        ostgp = ctx.enter_context(tc.tile_pool(name="ostgp", bufs=2))

        # ---- ACT table preload: dummy exp forces the single table-set load
        # at kernel start instead of mid-attention.
        warm = consts.tile([1, 32], F32, tag="warm")
        nc.vector.memset(warm, 1.0)
        nc.scalar.activation(out=warm, in_=warm, func=AF.Exp)

        # block-diagonal ones [128,128]: one K=128 matmul sums 64-feature
        # blocks of both pair halves (result broadcast across each half)
        bdiag = consts.tile([P, P], BF16, tag="bdiag")
        nc.vector.memset(bdiag, 0.0)
        nc.vector.memset(bdiag[0:64, 0:64], 1.0)
        nc.vector.memset(bdiag[64:128, 64:128], 1.0)

        vprime = consts.tile([P, NKc, HL, 2 * HD], BF16, tag="vprime")
        nc.vector.memset(vprime[:, :, :, HD:2 * HD], 1.0)

        # ---- input staging, split across the two HW DGE queues (sync and
        # scalar) so the 6.3MB input stream runs at ~2x one queue's ~200GB/s.
        # All scalar-queue DMAs are issued at the head, before the exp stream.
        # scalar (an HWDGE engine) is deliberately NOT used for input DMA:
        # its dispatch+ring waits would delay the exp stream by ~20us.
        # Every tensor is split kc-even/kc-odd across the sync and gpsimd
        # queues (each ~140GB/s) and ordered by first use, so each lands in
        # half the single-queue time.
        def eng2(kc):
            return nc.sync if kc % 2 == 0 else nc.gpsimd

        def load_w(dram, name):
            tiles = [consts.tile([P, nH], BF16, tag=f"w{name}{kc}", name=f"w{name}{kc}")
                     for kc in range(KC)]
            for kc in range(KC):
                eng2(kc).dma_start(out=tiles[kc], in_=dram.ap()[kc * P:(kc + 1) * P, :])
            return tiles

        def load_x(cb):
            c0 = cb * QB
            for kc in range(KC):
                eng2(kc).dma_start(out=xT_sb[kc][:, c0:c0 + QB],
                                   in_=xT_d.ap()[kc * P:(kc + 1) * P, c0:c0 + QB])

        ident = consts.tile([P, P], BF16, tag="ident")
        nc.sync.dma_start(out=ident, in_=id_d.ap())
        xT_sb = [consts.tile([P, S], BF16, tag=f"xT{kc}", name=f"xT{kc}") for kc in range(KC)]
        wk_sb = load_w(wk_d, "k")
        wq_sb = load_w(wq_d, "q")
        load_x(0)
        wv_sb = load_w(wv_d, "v")
        load_x(1)
        load_x(2)
        load_x(3)
        wo_sb = []
        for p in range(NP):
            t = consts.tile([P, D], BF16, tag=f"wo{p}", name=f"wo{p}")
            eng2(p).dma_start(out=t, in_=wo_d.ap()[p * P:(p + 1) * P, :])
            wo_sb.append(t)

        # ---- persistent feature-major tensors ----
        QT = [consts.tile([P, S], BF16, tag=f"QT{p}", name=f"QT{p}") for p in range(NP)]
        KT = [consts.tile([P, S], BF16, tag=f"KT{p}", name=f"KT{p}") for p in range(NP)]
        VT = [consts.tile([P, S], BF16, tag=f"VT{p}", name=f"VT{p}") for p in range(NP)]
        y_excl = [consts.tile([P, S], BF16, tag=f"yx{p}", name=f"yx{p}") for p in range(NP)]

        # ---- small task emitters (run as PE fillers, a ~0.5us step at a
        # time so ScalarE never waits behind a long PE filler burst) ----
        def proj_task(w_sb, dst, p, cb):
            """dst[p][:, cb*QB:(cb+1)*QB] = W_pair.T @ x chunk (K=128 x 8)."""
            ps = psF.tile([P, QB], F32, tag="f", name=f"pj{p}{cb}")
            for kc in range(KC):
                nc.tensor.matmul(
                    ps,
                    lhsT=w_sb[kc][:, p * P:(p + 1) * P],
                    rhs=xT_sb[kc][:, cb * QB:(cb + 1) * QB],
                    start=(kc == 0), stop=(kc == KC - 1))
                if kc % 2 == 1 and kc < KC - 1:
                    yield
            nc.vector.tensor_copy(out=dst[p][:, cb * QB:(cb + 1) * QB], in_=ps)

        def vprime_task(p, kc):
            """position-major V chunk via PE transpose of the feature-major
            VT block -- ~0.3us instead of an 8-matmul projection."""
            require(("V", p, kc // (NKc // NQ)))
            ps = psF.tile([P, P], BF16, tag="f", name=f"tr{p}{kc}")
            nc.tensor.transpose(ps, VT[p][:, kc * P:(kc + 1) * P], ident)
            nc.vector.tensor_copy(out=vprime[:, kc, 2 * p, 0:HD], in_=ps[:, 0:HD])
            nc.vector.tensor_copy(out=vprime[:, kc, 2 * p + 1, 0:HD],
                                  in_=ps[:, HD:2 * HD])
            return
            yield

        def e_task(qb, mt):
            """out-proj m-tile: K=128 per pair, accumulated over both pairs."""
            q0 = qb * QB
            if False:
                yield
            ps = psF.tile([P, QB], F32, tag="f", name=f"e{qb}{mt}")
            for p in range(NP):
                nc.tensor.matmul(
                    ps,
                    lhsT=wo_sb[p][:, mt * P:(mt + 1) * P],
                    rhs=y_excl[p][:, q0:q0 + QB],
                    start=(p == 0), stop=(p == NP - 1))
            ostg = ostgp.tile([P, QB], F32, tag="ostg", name="ostg")
            nc.vector.tensor_copy(out=ostg, in_=ps)
            eng2(mt).dma_start(
                out=outT_d.ap()[mt * P:(mt + 1) * P, q0:q0 + QB], in_=ostg)

        # filler task queue: generator tasks are advanced one ~0.5us step at
        # a time (pump, once per kc inside attention loops) or run to
        # completion on demand (require, when a consumer is about to be
        # emitted) -- emission order defines the dependencies Tile sees, so a
        # consumer must never precede its producer task.
        fillers = []
        tasks = {}

        def pump(n=1):
            for _ in range(n):
                while fillers:
                    key = fillers[0]
                    g = tasks.get(key)
                    if g is None:
                        fillers.pop(0)
                        continue
                    try:
                        next(g)
                    except StopIteration:
                        tasks.pop(key, None)
                        fillers.pop(0)
                    break

        def require(key):
            g = tasks.pop(key, None)
            if g is not None:
                for _ in g:
                    pass

        def add_task(key, gen_fn):
            tasks[key] = gen_fn()
            fillers.append(key)

        def run_task(gen_fn):
            for _ in gen_fn():
                pass

        # ---- D1: scores + exp + attn@V for one (q-block, pair) ----
        def emit_d1(qb, p, pending=None, slow_pump=False):
            """pending = deferred exclusive chain (qb', p', ysb, den) from the
            previous block, emitted a few kc in so its serial DVE chain and
            ones-matmuls overlap this block's scores instead of head-blocking
            the in-order PE queue at the boundary."""
            q0 = qb * QB
            KTp, QTp = KT[p], QT[p]
            yp = [psY.tile([HD * 2, QB], F32, tag="y", name=f"yp{p}{h}")
                  for h in range(2)]

            def attn_v(pT, kc):
                for h in range(2):
                    nc.tensor.matmul(
                        yp[h],
                        lhsT=vprime[:, kc, 2 * p + h, :],
                        rhs=pT[:, h * QB:(h + 1) * QB],
                        start=(kc == 0), stop=(kc == NKc - 1))

            require(("Q", p, qb))
            # prefetch the next block's projections mid-loop so its first
            # scores are never blocked on a cold 8-matmul require burst
            if qb + 1 < NQ:
                prefetch = [("Q", p, qb + 1)]
            elif p + 1 < NP:
                prefetch = [("K", p + 1, 0), ("K", p + 1, 1), ("K", p + 1, 2),
                            ("K", p + 1, 3), ("V", p + 1, 0), ("Q", p + 1, 0),
                            ("vp", p + 1, 0), ("vp", p + 1, 1)]
            else:
                prefetch = []
            prev = None
            for kc in range(NKc):
                require(("K", p, kc // (NKc // NQ)))
                require(("vp", p, kc))
                if kc >= 7 and prefetch:
                    require(prefetch.pop(0))
                sc = psS.tile([P, 2 * QB], F32, tag="s", name=f"sc{p}")
                # the two heads' K=64 score matmuls run concurrently on
                # disjoint PE row groups (lhsT/rhs at base 0 vs 64)
                for h in range(2):
                    nc.tensor.matmul(
                        sc[:, h * QB:(h + 1) * QB],
                        lhsT=KTp[h * HD:(h + 1) * HD, kc * P:(kc + 1) * P],
                        rhs=QTp[h * HD:(h + 1) * HD, q0:q0 + QB],
                        start=True, stop=True)
                pT = pP.tile([P, 2 * QB], BF16, tag="pt", name=f"pt{p}")
                nc.scalar.activation(out=pT, in_=sc, func=AF.Exp, scale=0.125)
                if prev is not None:
                    attn_v(*prev)
                prev = (pT, kc)
                if kc == 3 and pending is not None:
                    emit_d2(*pending)
                elif not (slow_pump and kc % 2 == 1):
                    pump(1)
            attn_v(*prev)

            ysb = ysbp.tile([P, QB], BF16, tag="ysb", name="ysb")
            den = denp.tile([P, QB], F32, tag="den", name="den")
            nc.vector.tensor_copy(out=ysb[0:64, :], in_=yp[0][0:64, :])
            nc.vector.tensor_copy(out=ysb[64:128, :], in_=yp[1][0:64, :])
            nc.vector.tensor_copy(out=den[0:64, :], in_=yp[0][64:128, :])
            nc.vector.tensor_copy(out=den[64:128, :], in_=yp[1][64:128, :])
            return ysb, den

        # ---- D2: pair-packed exclusive step ----
        def emit_d2(qb, p, ysb, den, tail=False):
            require(("V", p, qb))
            q0 = qb * QB
            vtp = VT[p][:, q0:q0 + QB]

            def tick():
                if tail:
                    pump(1)

            beta = betap.tile([P, QB], F32, tag="beta", name="beta")
            nc.vector.reciprocal_approx_fast(out=beta, in_=den)
            vsq = tmpa.tile([P, QB], BF16, tag="vsq", name="vsq")
            nc.vector.tensor_mul(vsq, vtp, vtp)
            d2B = psF.tile([P, QB], F32, tag="f", name="d2B")
            nc.tensor.matmul(d2B, lhsT=bdiag, rhs=vsq, start=True, stop=True)
            tick()
            r2 = r2p.tile([P, QB], F32, tag="r2", name="r2")
            nc.vector.reciprocal_approx_fast(out=r2, in_=d2B)
            t_yv = tmpa.tile([P, QB], BF16, tag="tyv", name="tyv")
            nc.vector.tensor_mul(t_yv, ysb, vtp)
            d1B = psF.tile([P, QB], F32, tag="f", name="d1B")
            nc.tensor.matmul(d1B, lhsT=bdiag, rhs=t_yv, start=True, stop=True)
            tick()
            aB = tmpb.tile([P, QB], BF16, tag="ab", name="ab")
            nc.vector.tensor_mul(aB, d1B, r2)
            tick()
            t2 = tmpc.tile([P, QB], BF16, tag="t2", name="t2")
            nc.vector.tensor_mul(t2, vtp, aB)
            tick()
            u = tmpb.tile([P, QB], BF16, tag="u", name="u")
            nc.vector.tensor_sub(u, ysb, t2)
            tick()
            nc.vector.tensor_mul(y_excl[p][:, q0:q0 + QB], u, beta)

        # ---- emission ----
        # upfront: keys/queries/values for pair0 q-block 0 and the first
        # vprime chunks; everything else queues as fillers (requires enforce
        # dependency order regardless of pump progress).
        run_task(lambda: proj_task(wk_sb, KT, 0, 0))
        run_task(lambda: proj_task(wq_sb, QT, 0, 0))
        run_task(lambda: proj_task(wv_sb, VT, 0, 0))
        run_task(lambda: vprime_task(0, 0))
        run_task(lambda: vprime_task(0, 1))
        for kc in range(2, 4):
            add_task(("vp", 0, kc), lambda kc=kc: vprime_task(0, kc))
        add_task(("K", 0, 1), lambda: proj_task(wk_sb, KT, 0, 1))
        for kc in range(4, 8):
            add_task(("vp", 0, kc), lambda kc=kc: vprime_task(0, kc))
        add_task(("K", 0, 2), lambda: proj_task(wk_sb, KT, 0, 2))
        add_task(("V", 0, 2), lambda: proj_task(wv_sb, VT, 0, 2))
        for kc in range(8, 12):
            add_task(("vp", 0, kc), lambda kc=kc: vprime_task(0, kc))
        add_task(("K", 0, 3), lambda: proj_task(wk_sb, KT, 0, 3))
        add_task(("V", 0, 3), lambda: proj_task(wv_sb, VT, 0, 3))
        for kc in range(12, NKc):
            add_task(("vp", 0, kc), lambda kc=kc: vprime_task(0, kc))
        add_task(("V", 0, 1), lambda: proj_task(wv_sb, VT, 0, 1))
        for cb in range(1, NQ):
            add_task(("Q", 0, cb), lambda cb=cb: proj_task(wq_sb, QT, 0, cb))
        for cb in range(NQ):
            add_task(("K", 1, cb), lambda cb=cb: proj_task(wk_sb, KT, 1, cb))
            add_task(("V", 1, cb), lambda cb=cb: proj_task(wv_sb, VT, 1, cb))
        for kc in range(NKc):
            add_task(("vp", 1, kc), lambda kc=kc: vprime_task(1, kc))
        for cb in range(NQ):
            add_task(("Q", 1, cb), lambda cb=cb: proj_task(wq_sb, QT, 1, cb))

        # pair 0 over all q-blocks, then pair 1; each block's exclusive chain
        # is deferred into the next block's kc loop; out-proj groups are
        # appended as fillers once both pairs of a q-block are done.
        pending = None
        for p in range(NP):
            for qb in range(NQ):
                ysb, den = emit_d1(qb, p, pending,
                                   slow_pump=(p == NP - 1 and qb == NQ - 1))
                if pending is not None and pending[1] == NP - 1:
                    for mt in range(D // P):
                        add_task(("e", pending[0], mt),
                                 lambda qb=pending[0], mt=mt: e_task(qb, mt))
                pending = (qb, p, ysb, den)
        emit_d2(*pending, tail=True)
        # drain remaining fillers, then the last q-block's out-proj
        while fillers:
            pump(1)
        for mt in range(D // P):
            run_task(lambda mt=mt: e_task(NQ - 1, mt))

    nc.finalize()
    return nc


def shard_inputs(x, Wq, bq, Wk, bk, Wv, bv, Wo, bo, n_cores=N_CORES):
    """Full inputs -> per-core input maps (host-side transpose/slice/reshape)."""
    assert not (np.any(bq) or np.any(bk) or np.any(bv)), "nonzero qkv bias unsupported"
    H = Wq.shape[1]
    cores_per_batch = n_cores // x.shape[0]
    hl = H // cores_per_batch
    bf = ml_dtypes.bfloat16
    in_maps = []
    for c in range(n_cores):
        b = c // cores_per_batch
        h0 = (c % cores_per_batch) * hl
        in_maps.append({
            "xT": np.ascontiguousarray(x[b].T).astype(bf),
            "wq": np.ascontiguousarray(Wq[:, h0:h0 + hl, :].reshape(Wq.shape[0], -1)).astype(bf),
            "wk": np.ascontiguousarray(Wk[:, h0:h0 + hl, :].reshape(Wk.shape[0], -1)).astype(bf),
            "wv": np.ascontiguousarray(Wv[:, h0:h0 + hl, :].reshape(Wv.shape[0], -1)).astype(bf),
            "wo": np.ascontiguousarray(Wo[h0:h0 + hl].reshape(-1, Wo.shape[2])).astype(bf),
            "ident": np.eye(128, dtype=bf),
        })
    return in_maps


_NC_CACHE = {}


def _get_nc():
    if "nc" not in _NC_CACHE:
        _NC_CACHE["nc"] = build_nc()
    return _NC_CACHE["nc"]


def run_sharded(inputs, trace=False, trace_cores=None):
    """Run the SPMD kernel; returns (full_output, BassKernelResults)."""
    x, bo = inputs["x"], inputs["bo"]
    nc = _get_nc()
    in_maps = shard_inputs(**inputs)
    res = bass_utils.run_bass_kernel_spmd(
        nc, in_maps, core_ids=list(range(N_CORES)),
        trace=trace, trace_cores=trace_cores)
    cores_per_batch = N_CORES // x.shape[0]
    out = np.empty_like(x)
    for b in range(x.shape[0]):
        acc = np.zeros((x.shape[2], x.shape[1]), np.float32)
        for c in range(b * cores_per_batch, (b + 1) * cores_per_batch):
            acc += res.results[c]["outT"]
        out[b] = acc.T + bo[None, :]
    return out, res


def kernel(**inputs):
    out, _ = run_sharded(inputs)
    return out


# revision 12
# speedup vs baseline: 1.4387x; 1.0737x over previous
"""Multi-head attention with exclusive post-processing, sharded over 8 trn2 cores.

Sharding: data-parallel over batch (2) x tensor-parallel over heads (16 -> 4/core).
Each core computes a partial transposed output [D, S] for its batch from its 4
heads; the host sums the 4 partials per batch, transposes back, and adds bo.

Per-core layout (feature-major "T" = [feature, position]); heads processed as
PAIRS stacked on the partition axis so DVE/ACT work runs at full 128-lane width:
  QT/KT/VT [128, S]  per pair (bf16)
  vprime   [128 pos, kc, h, v|ones]  position-major V with a 64-wide ones block
  scores: the two heads of a pair run as CONCURRENT K=64 matmuls on disjoint
    PE row groups (tile_position (0,0)/(64,0)) into one [128, 2*512] PSUM tile,
    so one [128,1024] exp covers both heads. ScalarE (the critical engine at
    ~147us of exp) sees the same element count as per-head processing; the PE
    sees half the score cost.
  attn@V: lhsT = [v | ones] per head -> rows 0:64 unnormalized Y, rows 64:128
    softmax denominator (free: PE matmul cost is independent of output rows).
  exclusive step, pair-packed: y_excl = (Y - (Y.v)/(sum v^2) v) / denom with
    both reciprocals on DVE (reciprocal_approx_fast, ~51 ULP) instead of
    exp(-ln(x)) on the busy ScalarE. Pair sums via one block-diagonal ones
    matmul (K=128).
  out-proj: per-pair K=128 contraction (wo stacked [128, D]).

Emission order: kc-streamed projections start as DMA chunks land; a queue of
small PE "filler" tasks (remaining projections, vprime chunks, out-proj
groups) is pumped one per kc inside the attention loops so the PE never idles
long enough for the HAM clock gate to re-throttle, and ScalarE stays fed.
"""

from contextlib import ExitStack

import ml_dtypes
import numpy as np

import concourse.mybir as mybir
import concourse.tile as tile
from concourse import bacc, bass_utils

F32 = mybir.dt.float32
BF16 = mybir.dt.bfloat16
AF = mybir.ActivationFunctionType

B, S_FULL, D_FULL, H_FULL = 2, 2048, 1024, 16
HD = 64
N_CORES = 8
HEADS_PER_CORE = H_FULL * B // N_CORES  # 4


def build_nc(S=S_FULL, D=D_FULL, HL=HEADS_PER_CORE):
    P = 128
    nH = HL * HD          # local fused head dim (256)
    KC = D // P           # x contraction chunks (8)
    NKc = S // P          # key chunks (16)
    QB = 512              # q block (one PSUM bank per head)
    NQ = S // QB          # 4
    NP = HL // 2          # head pairs (2)
    XH = S // 2           # x DMA column half

    nc = bacc.Bacc(None, target_bir_lowering=False)

    xT_d = nc.dram_tensor("xT", [D, S], BF16, kind="ExternalInput")
    wq_d = nc.dram_tensor("wq", [D, nH], BF16, kind="ExternalInput")
    wk_d = nc.dram_tensor("wk", [D, nH], BF16, kind="ExternalInput")
    wv_d = nc.dram_tensor("wv", [D, nH], BF16, kind="ExternalInput")
    wo_d = nc.dram_tensor("wo", [nH, D], BF16, kind="ExternalInput")
    id_d = nc.dram_tensor("ident", [P, P], BF16, kind="ExternalInput")
    outT_d = nc.dram_tensor("outT", [D, S], F32, kind="ExternalOutput")

    with tile.TileContext(nc) as tc, ExitStack() as ctx:
        consts = ctx.enter_context(tc.tile_pool(name="consts", bufs=1))
        psS = ctx.enter_context(tc.tile_pool(name="psS", bufs=2, space="PSUM"))
        psY = ctx.enter_context(tc.tile_pool(name="psY", bufs=2, space="PSUM"))
        psF = ctx.enter_context(tc.tile_pool(name="psF", bufs=2, space="PSUM"))
        pP = ctx.enter_context(tc.tile_pool(name="pP", bufs=5))
        ysbp = ctx.enter_context(tc.tile_pool(name="ysbp", bufs=3))
        denp = ctx.enter_context(tc.tile_pool(name="denp", bufs=3))
        betap = ctx.enter_context(tc.tile_pool(name="betap", bufs=2))
        r2p = ctx.enter_context(tc.tile_pool(name="r2p", bufs=2))
        tmpa = ctx.enter_context(tc.tile_pool(name="tmpa", bufs=2))
        tmpb = ctx.enter_context(tc.tile_pool(name="tmpb", bufs=2))
        tmpc = ctx.enter_context(tc.tile_pool(name="tmpc", bufs=2))
        ostgp = ctx.enter_context(tc.tile_pool(name="ostgp", bufs=4))

        # ---- ACT table preload: dummy exp forces the single table-set load
        # at kernel start instead of mid-attention.
        warm = consts.tile([1, 32], F32, tag="warm")
        nc.vector.memset(warm, 1.0)
        nc.scalar.activation(out=warm, in_=warm, func=AF.Exp)

        # block-diagonal ones [128,128]: one K=128 matmul sums 64-feature
        # blocks of both pair halves (result broadcast across each half)
        bdiag = consts.tile([P, P], BF16, tag="bdiag")
        nc.vector.memset(bdiag, 0.0)
        nc.vector.memset(bdiag[0:64, 0:64], 1.0)
        nc.vector.memset(bdiag[64:128, 64:128], 1.0)

        vprime = consts.tile([P, NKc, HL, 2 * HD], BF16, tag="vprime")
        nc.vector.memset(vprime[:, :, :, HD:2 * HD], 1.0)

        # ---- input staging, split across the two HW DGE queues (sync and
        # scalar) so the 6.3MB input stream runs at ~2x one queue's ~200GB/s.
        # All scalar-queue DMAs are issued at the head, before the exp stream.
        # scalar (an HWDGE engine) is deliberately NOT used for input DMA:
        # its dispatch+ring waits would delay the exp stream by ~20us.
        # Every tensor is split kc-even/kc-odd across the sync and gpsimd
        # queues (each ~140GB/s) and ordered by first use, so each lands in
        # half the single-queue time.
        def eng2(kc):
            return nc.sync if kc % 2 == 0 else nc.gpsimd

        def load_w(dram, name):
            tiles = [consts.tile([P, nH], BF16, tag=f"w{name}{kc}", name=f"w{name}{kc}")
                     for kc in range(KC)]
            for kc in range(KC):
                eng2(kc).dma_start(out=tiles[kc], in_=dram.ap()[kc * P:(kc + 1) * P, :])
            return tiles

        def load_x(cb):
            c0 = cb * QB
            for kc in range(KC):
                eng2(kc).dma_start(out=xT_sb[kc][:, c0:c0 + QB],
                                   in_=xT_d.ap()[kc * P:(kc + 1) * P, c0:c0 + QB])

        ident = consts.tile([P, P], BF16, tag="ident")
        nc.sync.dma_start(out=ident, in_=id_d.ap())
        xT_sb = [consts.tile([P, S], BF16, tag=f"xT{kc}", name=f"xT{kc}") for kc in range(KC)]
        wk_sb = load_w(wk_d, "k")
        wq_sb = load_w(wq_d, "q")
        load_x(0)
        wv_sb = load_w(wv_d, "v")
        load_x(1)
        load_x(2)
        load_x(3)
        wo_sb = []
        for p in range(NP):
            t = consts.tile([P, D], BF16, tag=f"wo{p}", name=f"wo{p}")
            eng2(p).dma_start(out=t, in_=wo_d.ap()[p * P:(p + 1) * P, :])
            wo_sb.append(t)

        # ---- persistent feature-major tensors ----
        QT = [consts.tile([P, S], BF16, tag=f"QT{p}", name=f"QT{p}") for p in range(NP)]
        KT = [consts.tile([P, S], BF16, tag=f"KT{p}", name=f"KT{p}") for p in range(NP)]
        VT = [consts.tile([P, S], BF16, tag=f"VT{p}", name=f"VT{p}") for p in range(NP)]
        y_excl = [consts.tile([P, S], BF16, tag=f"yx{p}", name=f"yx{p}") for p in range(NP)]

        # ---- small task emitters (run as PE fillers, a ~0.5us step at a
        # time so ScalarE never waits behind a long PE filler burst) ----
        def proj_task(w_sb, dst, p, cb):
            """dst[p][:, cb*QB:(cb+1)*QB] = W_pair.T @ x chunk (K=128 x 8)."""
            ps = psF.tile([P, QB], F32, tag="f", name=f"pj{p}{cb}")
            for kc in range(KC):
                nc.tensor.matmul(
                    ps,
                    lhsT=w_sb[kc][:, p * P:(p + 1) * P],
                    rhs=xT_sb[kc][:, cb * QB:(cb + 1) * QB],
                    start=(kc == 0), stop=(kc == KC - 1))
                if kc % 2 == 1 and kc < KC - 1:
                    yield
            nc.vector.tensor_copy(out=dst[p][:, cb * QB:(cb + 1) * QB], in_=ps)

        def vprime_task(p, kc):
            """position-major V chunk via PE transpose of the feature-major
            VT block -- ~0.3us instead of an 8-matmul projection."""
            require(("V", p, kc // (NKc // NQ)))
            ps = psF.tile([P, P], BF16, tag="f", name=f"tr{p}{kc}")
            nc.tensor.transpose(ps, VT[p][:, kc * P:(kc + 1) * P], ident)
            nc.vector.tensor_copy(out=vprime[:, kc, 2 * p, 0:HD], in_=ps[:, 0:HD])
            nc.vector.tensor_copy(out=vprime[:, kc, 2 * p + 1, 0:HD],
                                  in_=ps[:, HD:2 * HD])
            return
            yield

        def e_task(qb, mt):
            """out-proj m-tile: K=128 per pair, accumulated over both pairs."""
            q0 = qb * QB
            if False:
                yield
            ps = psF.tile([P, QB], F32, tag="f", name=f"e{qb}{mt}")
            for p in range(NP):
                nc.tensor.matmul(
                    ps,
                    lhsT=wo_sb[p][:, mt * P:(mt + 1) * P],
                    rhs=y_excl[p][:, q0:q0 + QB],
                    start=(p == 0), stop=(p == NP - 1))
            ostg = ostgp.tile([P, QB], F32, tag="ostg", name="ostg")
            nc.vector.tensor_copy(out=ostg, in_=ps)
            eng2(mt).dma_start(
                out=outT_d.ap()[mt * P:(mt + 1) * P, q0:q0 + QB], in_=ostg)

        # filler task queue: generator tasks are advanced one ~0.5us step at
        # a time (pump, once per kc inside attention loops) or run to
        # completion on demand (require, when a consumer is about to be
        # emitted) -- emission order defines the dependencies Tile sees, so a
        # consumer must never precede its producer task.
        fillers = []
        tasks = {}

        def pump(n=1):
            for _ in range(n):
                while fillers:
                    key = fillers[0]
                    g = tasks.get(key)
                    if g is None:
                        fillers.pop(0)
                        continue
                    try:
                        next(g)
                    except StopIteration:
                        tasks.pop(key, None)
                        fillers.pop(0)
                    break

        def require(key):
            g = tasks.pop(key, None)
            if g is not None:
                for _ in g:
                    pass

        def mark_done(*keys):
            for k in keys:
                tasks[k] = None

        def add_task(key, gen_fn):
            tasks[key] = gen_fn()
            fillers.append(key)

        def run_task(gen_fn):
            for _ in gen_fn():
                pass

        # ---- D1: scores + exp + attn@V for one (q-block, pair) ----
        def emit_d1(qb, p, pending=None):
            """pending = deferred exclusive chain (qb', p', ysb, den) from the
            previous block, emitted a few kc in so its serial DVE chain and
            ones-matmuls overlap this block's scores instead of head-blocking
            the in-order PE queue at the boundary."""
            q0 = qb * QB
            KTp, QTp = KT[p], QT[p]
            yp = [psY.tile([HD * 2, QB], F32, tag="y", name=f"yp{p}{h}")
                  for h in range(2)]

            def attn_v(pT, kc):
                for h in range(2):
                    nc.tensor.matmul(
                        yp[h],
                        lhsT=vprime[:, kc, 2 * p + h, :],
                        rhs=pT[:, h * QB:(h + 1) * QB],
                        start=(kc == 0), stop=(kc == NKc - 1))

            require(("Q", p, qb))
            # prefetch the next block's projections mid-loop so its first
            # scores are never blocked on a cold 8-matmul require burst
            if qb + 1 < NQ:
                prefetch = [("Q", p, qb + 1)]
            elif p + 1 < NP:
                prefetch = [("K", p + 1, 0), ("K", p + 1, 1), ("K", p + 1, 2),
                            ("K", p + 1, 3), ("V", p + 1, 0), ("Q", p + 1, 0),
                            ("vp", p + 1, 0), ("vp", p + 1, 1)]
            else:
                prefetch = []
            # attn@V trails the exp stream by LAG chunks: the new block's
            # first attn@V (which must wait for the previous yp tiles to be
            # copied out) then never head-blocks the next scores in the
            # in-order PE queue.
            LAG = 3
            back = []
            for kc in range(NKc):
                require(("K", p, kc // (NKc // NQ)))
                require(("vp", p, kc))
                if kc >= 7 and prefetch:
                    require(prefetch.pop(0))
                sc = psS.tile([P, 2 * QB], F32, tag="s", name=f"sc{p}")
                # the two heads' K=64 score matmuls run concurrently on
                # disjoint PE row groups (lhsT/rhs at base 0 vs 64)
                for h in range(2):
                    nc.tensor.matmul(
                        sc[:, h * QB:(h + 1) * QB],
                        lhsT=KTp[h * HD:(h + 1) * HD, kc * P:(kc + 1) * P],
                        rhs=QTp[h * HD:(h + 1) * HD, q0:q0 + QB],
                        start=True, stop=True)
                pT = pP.tile([P, 2 * QB], BF16, tag="pt", name=f"pt{p}")
                nc.scalar.activation(out=pT, in_=sc, func=AF.Exp, scale=0.125)
                back.append((pT, kc))
                if len(back) > LAG:
                    attn_v(*back.pop(0))
                if kc == 3 and pending is not None:
                    emit_d2(*pending)
                else:
                    pump(1)
            for b in back:
                attn_v(*b)

            ysb = ysbp.tile([P, QB], BF16, tag="ysb", name="ysb")
            den = denp.tile([P, QB], F32, tag="den", name="den")
            nc.vector.tensor_copy(out=ysb[0:64, :], in_=yp[0][0:64, :])
            nc.vector.tensor_copy(out=ysb[64:128, :], in_=yp[1][0:64, :])
            nc.vector.tensor_copy(out=den[0:64, :], in_=yp[0][64:128, :])
            nc.vector.tensor_copy(out=den[64:128, :], in_=yp[1][64:128, :])
            return ysb, den

        # ---- D2: pair-packed exclusive step ----
        def emit_d2(qb, p, ysb, den, tail=False):
            require(("V", p, qb))
            q0 = qb * QB
            vtp = VT[p][:, q0:q0 + QB]

            def tick():
                if tail:
                    pump(1)

            beta = betap.tile([P, QB], F32, tag="beta", name="beta")
            nc.vector.reciprocal_approx_fast(out=beta, in_=den)
            vsq = tmpa.tile([P, QB], BF16, tag="vsq", name="vsq")
            nc.vector.tensor_mul(vsq, vtp, vtp)
            d2B = psF.tile([P, QB], F32, tag="f", name="d2B")
            nc.tensor.matmul(d2B, lhsT=bdiag, rhs=vsq, start=True, stop=True)
            tick()
            r2 = r2p.tile([P, QB], F32, tag="r2", name="r2")
            nc.vector.reciprocal_approx_fast(out=r2, in_=d2B)
            t_yv = tmpa.tile([P, QB], BF16, tag="tyv", name="tyv")
            nc.vector.tensor_mul(t_yv, ysb, vtp)
            d1B = psF.tile([P, QB], F32, tag="f", name="d1B")
            nc.tensor.matmul(d1B, lhsT=bdiag, rhs=t_yv, start=True, stop=True)
            tick()
            aB = tmpb.tile([P, QB], BF16, tag="ab", name="ab")
            nc.vector.tensor_mul(aB, d1B, r2)
            tick()
            t2 = tmpc.tile([P, QB], BF16, tag="t2", name="t2")
            nc.vector.tensor_mul(t2, vtp, aB)
            tick()
            u = tmpb.tile([P, QB], BF16, tag="u", name="u")
            nc.vector.tensor_sub(u, ysb, t2)
            tick()
            nc.vector.tensor_mul(y_excl[p][:, q0:q0 + QB], u, beta)

        # ---- emission ----
        # upfront: K/Q/V for pair0 q-block 0, interleaved per x-chunk so the
        # PE tracks the arriving DMA stream (and warms the clock gate) instead
        # of idling then bursting; then the first vprime chunks.
        psK = psF.tile([P, QB], F32, tag="f", name="pjK")
        psQ = psF.tile([P, QB], F32, tag="f", name="pjQ")
        psV = psS.tile([P, 2 * QB], F32, tag="s", name="pjV")
        for kc in range(KC):
            for w_sb, ps in ((wk_sb, psK), (wq_sb, psQ), (wv_sb, psV[:, 0:QB])):
                nc.tensor.matmul(ps, lhsT=w_sb[kc][:, 0:P],
                                 rhs=xT_sb[kc][:, 0:QB],
                                 start=(kc == 0), stop=(kc == KC - 1))
        nc.vector.tensor_copy(out=KT[0][:, 0:QB], in_=psK)
        nc.vector.tensor_copy(out=QT[0][:, 0:QB], in_=psQ)
        nc.vector.tensor_copy(out=VT[0][:, 0:QB], in_=psV[:, 0:QB])
        mark_done(("K", 0, 0), ("Q", 0, 0), ("V", 0, 0))
        run_task(lambda: vprime_task(0, 0))
        run_task(lambda: vprime_task(0, 1))
        for kc in range(2, 4):
            add_task(("vp", 0, kc), lambda kc=kc: vprime_task(0, kc))
        add_task(("K", 0, 1), lambda: proj_task(wk_sb, KT, 0, 1))
        for kc in range(4, 8):
            add_task(("vp", 0, kc), lambda kc=kc: vprime_task(0, kc))
        add_task(("K", 0, 2), lambda: proj_task(wk_sb, KT, 0, 2))
        add_task(("V", 0, 2), lambda: proj_task(wv_sb, VT, 0, 2))
        for kc in range(8, 12):
            add_task(("vp", 0, kc), lambda kc=kc: vprime_task(0, kc))
        add_task(("K", 0, 3), lambda: proj_task(wk_sb, KT, 0, 3))
        add_task(("V", 0, 3), lambda: proj_task(wv_sb, VT, 0, 3))
        for kc in range(12, NKc):
            add_task(("vp", 0, kc), lambda kc=kc: vprime_task(0, kc))
        add_task(("V", 0, 1), lambda: proj_task(wv_sb, VT, 0, 1))
        for cb in range(1, NQ):
            add_task(("Q", 0, cb), lambda cb=cb: proj_task(wq_sb, QT, 0, cb))
        for cb in range(NQ):
            add_task(("K", 1, cb), lambda cb=cb: proj_task(wk_sb, KT, 1, cb))
            add_task(("V", 1, cb), lambda cb=cb: proj_task(wv_sb, VT, 1, cb))
        for kc in range(NKc):
            add_task(("vp", 1, kc), lambda kc=kc: vprime_task(1, kc))
        for cb in range(NQ):
            add_task(("Q", 1, cb), lambda cb=cb: proj_task(wq_sb, QT, 1, cb))

        # pair 0 over all q-blocks, then pair 1; each block's exclusive chain
        # is deferred into the next block's kc loop; out-proj groups are
        # appended as fillers once both pairs of a q-block are done.
        pending = None
        for p in range(NP):
            for qb in range(NQ):
                ysb, den = emit_d1(qb, p, pending)
                if pending is not None and pending[1] == NP - 1:
                    for mt in range(D // P):
                        add_task(("e", pending[0], mt),
                                 lambda qb=pending[0], mt=mt: e_task(qb, mt))
                pending = (qb, p, ysb, den)
        # tail: the last q-block's out-proj m-tiles 0..5 accumulate in the
        # freed scores/yp PSUM banks; their pair-0 contributions keep the PE
        # dense (HAM stays warm) while the final exclusive chain runs on DVE
        # (psF is left to the chain's two ones-matmuls).
        while fillers:
            pump(1)
        qL = NQ - 1
        q0 = qL * QB
        big = [psS.tile([P, 2 * QB], F32, tag="s", name=f"et{i}") for i in range(2)]
        ev = ([big[0][:, 0:QB], big[0][:, QB:2 * QB],
               big[1][:, 0:QB], big[1][:, QB:2 * QB]]
              + [psY.tile([HD * 2, QB], F32, tag="y", name=f"ey{i}") for i in range(2)])

        def tail_e(p_idx, stop):
            for mt in range(6):
                nc.tensor.matmul(
                    ev[mt],
                    lhsT=wo_sb[p_idx][:, mt * P:(mt + 1) * P],
                    rhs=y_excl[p_idx][:, q0:q0 + QB],
                    start=(p_idx == 0), stop=stop)

        tail_e(0, False)
        emit_d2(*pending, tail=True)
        tail_e(1, True)
        for mt in range(6):
            ostg = ostgp.tile([P, QB], F32, tag="ostg", name="ostg")
            nc.vector.tensor_copy(out=ostg, in_=ev[mt])
            eng2(mt).dma_start(
                out=outT_d.ap()[mt * P:(mt + 1) * P, q0:q0 + QB], in_=ostg)
        for mt in range(6, D // P):
            run_task(lambda mt=mt: e_task(qL, mt))

    nc.finalize()
    return nc


def shard_inputs(x, Wq, bq, Wk, bk, Wv, bv, Wo, bo, n_cores=N_CORES):
    """Full inputs -> per-core input maps (host-side transpose/slice/reshape)."""
    assert not (np.any(bq) or np.any(bk) or np.any(bv)), "nonzero qkv bias unsupported"
    H = Wq.shape[1]
    cores_per_batch = n_cores // x.shape[0]
    hl = H // cores_per_batch
    bf = ml_dtypes.bfloat16
    in_maps = []
    for c in range(n_cores):
        b = c // cores_per_batch
        h0 = (c % cores_per_batch) * hl
        in_maps.append({
            "xT": np.ascontiguousarray(x[b].T).astype(bf),
            "wq": np.ascontiguousarray(Wq[:, h0:h0 + hl, :].reshape(Wq.shape[0], -1)).astype(bf),
            "wk": np.ascontiguousarray(Wk[:, h0:h0 + hl, :].reshape(Wk.shape[0], -1)).astype(bf),
            "wv": np.ascontiguousarray(Wv[:, h0:h0 + hl, :].reshape(Wv.shape[0], -1)).astype(bf),
            "wo": np.ascontiguousarray(Wo[h0:h0 + hl].reshape(-1, Wo.shape[2])).astype(bf),
            "ident": np.eye(128, dtype=bf),
        })
    return in_maps


_NC_CACHE = {}


def _get_nc():
    if "nc" not in _NC_CACHE:
        _NC_CACHE["nc"] = build_nc()
    return _NC_CACHE["nc"]


def run_sharded(inputs, trace=False, trace_cores=None):
    """Run the SPMD kernel; returns (full_output, BassKernelResults)."""
    x, bo = inputs["x"], inputs["bo"]
    nc = _get_nc()
    in_maps = shard_inputs(**inputs)
    res = bass_utils.run_bass_kernel_spmd(
        nc, in_maps, core_ids=list(range(N_CORES)),
        trace=trace, trace_cores=trace_cores)
    cores_per_batch = N_CORES // x.shape[0]
    out = np.empty_like(x)
    for b in range(x.shape[0]):
        acc = np.zeros((x.shape[2], x.shape[1]), np.float32)
        for c in range(b * cores_per_batch, (b + 1) * cores_per_batch):
            acc += res.results[c]["outT"]
        out[b] = acc.T + bo[None, :]
    return out, res


def kernel(**inputs):
    out, _ = run_sharded(inputs)
    return out
